# revision 17
# baseline (speedup 1.0000x reference)
"""GAT (2-layer, heads=1) on 8 Trainium2 NeuronCores.

Strategy (1D node partition):
  - Nodes are split into 8 chunks of NL; core c owns dst-chunk c.
  - Layer tables h^T/alpha_src are computed per-chunk (feature-major via
    PE matmuls on x^T), AllGathered, and kept in SBUF as a per-partition-group
    table: partition group g (16 partitions) holds (h|alpha_src) of chunk g,
    packed two fp16 per fp32 word.
  - Edges are bucketed on the host by (dst-core, src-chunk, dst-block) and
    sorted by dst. Each edge tile covers one block of NB dst nodes for all
    8 src-chunk groups at once; per-edge h|alpha_src gathers run on GPSIMD
    (ap_gather), per-edge math on DVE/ACT, and segment-sums via masked
    tensor_tensor_scan (state = mask*state + w, mask=0 at segment starts)
    + a boundary gather at each node's last edge. Group partials are
    combined with a matmul against a 0/1 selection matrix; softmax
    normalization divides at the node level.
  - Per-edge alpha_dst values are NOT gathered. Instead, for each tile a
    GPSIMD local_scatter places each dst-slot's alpha_dst (fp16) at that
    slot's first-edge stream position (host-known structure; per-partition
    independent indices; ~3.4us per 512-idx scatter vs ~64us for the old
    per-edge ap_gather), and a masked scan broadcasts the value along the
    run. The run-start mask itself is host-precomputed structure, DMA'd
    per tile (HWDGE), freeing GPSIMD entirely.
  - local_scatter and ap_gather live in different GPSIMD IRAM libraries, so
    each layer runs all local_scatters in a "seed phase" (results staged in
    DRAM), then all ap_gathers in the edge phase, with a scheduler fence
    between them -> 2 library reloads per layer instead of per-tile thrash.
  - Slot 0 of every bucket is a sentinel; nodes with no edges in a bucket
    point their boundary index at slot 0 (whose running sum is always 0).
  - Softmax max-subtraction is skipped: attention logits here are O(1), and
    alpha = exp(e)/sum(exp(e)) is shift-invariant.

Host preprocessing only reorders/buckets edge indices and emits 0/1 masks
(structure), never touches float data dependent on device results.
"""

import math

import ml_dtypes
import numpy as np

from concourse import bass, bacc, mybir
import concourse.tile as tile

F32 = mybir.dt.float32
BF16 = mybir.dt.bfloat16
F16 = mybir.dt.float16
I16 = mybir.dt.int16
I32 = mybir.dt.int32

NEG_SLOPE = 0.2
# fp32 word 0xF7500000: low fp16 lane (h) = 0, high fp16 lane
# (alpha_src) = -29952 -> exp(lrelu(...)) == 0, so sentinel edges vanish.
SENT_PACKED = float(
    np.frombuffer(
        (np.uint32(np.float16(-29952.0).view(np.uint16)) << np.uint32(16)).tobytes(),
        np.float32,
    )[0]
)

FULL_CFG = dict(
    NCORES=8, N=100000, F=512, H=16,
    NL=12500, NB=500, NT=25, CH=500, NCH=25,
)

LS_MAX = 2046   # local_scatter num_elems limit (num_elems*32 < 2^16)


def _halves(T_e):
    nh = math.ceil(T_e / LS_MAX)
    w0 = (T_e + nh - 1) // nh
    w0 = (w0 + 1) // 2 * 2
    return nh, w0


# ---------------------------------------------------------------- host prep

def _round_up(x, m):
    return (x + m - 1) // m * m


def host_prep(edge_index, cfg):
    """Bucket and sort edges; build device index streams and masks
    (structure only, no float data).

    Returns (T_e, per_core) where per_core[c] has:
      'idxs'  [128, NT*(T16+32)] i16   (src-local gather idx + boundary idx)
      'maskp' [128, NT*T_e]      f16   (1 inside a run, 0 at run starts)
      'seedp' [128, NT*NH*512]   i16   (run-start positions per dst slot,
                                        split into NH stream halves; -1 = none)
    """
    NC, N, NL = cfg["NCORES"], cfg["N"], cfg["NL"]
    NB, NT = cfg["NB"], cfg["NT"]
    G = 8

    # Self-loops are NOT added here: their contribution is computed
    # analytically at the node level on-device (no gather needed).
    src = np.asarray(edge_index[0], dtype=np.int64)
    dst = np.asarray(edge_index[1], dtype=np.int64)

    core = dst // NL
    grp = src // NL
    order = np.lexsort((src, dst, grp, core))
    src, dst, core, grp = (a[order] for a in (src, dst, core, grp))
    blk = (dst % NL) // NB

    bucket = (core * G + grp) * NT + blk
    nbuck = NC * G * NT
    counts = np.bincount(bucket, minlength=nbuck).reshape(NC, G, NT)
    # +1: slot 0 of every bucket is a sentinel (known-zero running sum).
    # Per-tile stream width: max over (core, group) for that tile index
    # (SPMD: all cores share one program, so width is the cross-core max).
    T_list = tuple(_round_up(int(counts[:, :, t].max()) + 17, 16)
                   for t in range(NT))
    assert max(T_list) <= 32767
    import os
    if os.environ.get("UNIFORM_T"):
        T_list = (_round_up(max(T_list), int(os.environ["UNIFORM_T"])),) * NT
    T_max = max(T_list)

    starts = np.zeros(nbuck, dtype=np.int64)
    starts[1:] = np.cumsum(counts.reshape(-1))[:-1]
    pos = np.arange(src.size) - starts[bucket] + 1

    is_last = np.ones(src.size, dtype=bool)
    is_last[:-1] = ~((bucket[1:] == bucket[:-1]) & (dst[1:] == dst[:-1]))
    is_first = np.ones(src.size, dtype=bool)
    is_first[1:] = ~((bucket[1:] == bucket[:-1]) & (dst[1:] == dst[:-1]))

    srcl = np.full((NC, G, NT, T_max), NL, dtype=np.int16)
    bnd = np.zeros((NC, G, NT, 512), dtype=np.int16)  # default: sentinel slot 0
    mask = np.ones((NC, G, NT, T_max), dtype=np.float16)
    mask[..., 0] = 0.0
    # seeds per tile half, ragged over tiles; NH_t halves of width W0_t
    seed_blocks = [np.full((NC, G, _halves(T_list[t])[0], 512), -1, np.int16)
                   for t in range(NT)]

    c_, g_, b_ = core, grp, blk
    dloc = (dst % NL) % NB
    srcl[c_, g_, b_, pos] = (src % NL).astype(np.int16)
    bnd[c_[is_last], g_[is_last], b_[is_last],
        dloc[is_last]] = pos[is_last].astype(np.int16)
    mask[c_[is_first], g_[is_first], b_[is_first], pos[is_first]] = 0.0
    for t in range(NT):
        selt = is_first & (blk == t)
        p1 = pos[selt]
        W0t = _halves(T_list[t])[1]
        hh = (p1 // W0t).astype(np.int64)
        seed_blocks[t][c_[selt], g_[selt], hh,
                       dloc[selt]] = (p1 - hh * W0t).astype(np.int16)

    def wrap16(a):
        # [NC, G, w] -> [NC, 128, w//16]; w16[c, 16g+p, s] = a[c, g, s*16+p]
        n = a.shape[-1]
        return (a.reshape(NC, G, n // 16, 16)
                 .transpose(0, 1, 3, 2)
                 .reshape(NC, 128, n // 16))

    idx_parts, mask_parts, seed_parts = [], [], []
    for t in range(NT):
        Tt = T_list[t]
        idx_parts.append(wrap16(srcl[:, :, t, :Tt]))
        idx_parts.append(wrap16(bnd[:, :, t, :]))
        mask_parts.append(np.repeat(mask[:, :, t, :Tt], 16, axis=1))
        sb = seed_blocks[t]
        seed_parts.append(np.repeat(
            sb.reshape(NC, G, -1), 16, axis=1))
    idxs = np.ascontiguousarray(np.concatenate(idx_parts, axis=2))
    maskw = np.ascontiguousarray(np.concatenate(mask_parts, axis=2))
    seedw = np.ascontiguousarray(np.concatenate(seed_parts, axis=2))

    per_core = [{"idxs": idxs[c], "maskp": maskw[c], "seedp": seedw[c]}
                for c in range(NC)]
    return T_list, per_core


# ------------------------------------------------------------- device build

def build_nc(cfg, T_list, max_waits=2, ctrl_max_waits=1, split=True):
    NC, N, F, H = cfg["NCORES"], cfg["N"], cfg["F"], cfg["H"]
    NL, NB, NT = cfg["NL"], cfg["NB"], cfg["NT"]
    CH, NCH = cfg["CH"], cfg["NCH"]
    KB = F // 128
    NLP = NL + 16               # table width incl. sentinel
    T_max = max(T_list)
    # per-tile offsets into the concatenated idx / mask / seed streams
    idx_off, mask_off, seed_off = [], [], []
    io = mo = so = 0
    for t in range(NT):
        Tt = T_list[t]
        idx_off.append(io); mask_off.append(mo); seed_off.append(so)
        io += Tt // 16 + 32
        mo += Tt
        so += _halves(Tt)[0] * 512
    IWT, MWT, SWT = io, mo, so
    NBLK = math.ceil(NL / 128)
    NL2 = NBLK * 128
    W2 = NL2 // 2               # fp32 words backing the bf16 out1T row
    rg = [list(range(NC))]

    nc = bacc.Bacc("TRN2", target_bir_lowering=False)

    xtw = nc.declare_dram_parameter("xtw", [128, NCH * KB * CH], BF16, isOutput=False)
    w1 = nc.declare_dram_parameter("w1", [F, H], F32, isOutput=False)
    w2 = nc.declare_dram_parameter("w2", [H, H], F32, isOutput=False)
    a1rep = nc.declare_dram_parameter("a1rep", [H, 16], F32, isOutput=False)
    ad1rep = nc.declare_dram_parameter("ad1rep", [H, 16], F32, isOutput=False)
    a2rep = nc.declare_dram_parameter("a2rep", [H, 16], F32, isOutput=False)
    ad2rep = nc.declare_dram_parameter("ad2rep", [H, 16], F32, isOutput=False)
    b1p = nc.declare_dram_parameter("b1p", [H, 1], F32, isOutput=False)
    b2p = nc.declare_dram_parameter("b2p", [H, 1], F32, isOutput=False)
    selp = nc.declare_dram_parameter("selp", [128, 16], F32, isOutput=False)
    identp = nc.declare_dram_parameter("identp", [16, 16], F32, isOutput=False)
    idxsp = nc.declare_dram_parameter("idxs", [128, IWT], I16, isOutput=False)
    maskp = nc.declare_dram_parameter("maskp", [128, MWT], F16, isOutput=False)
    seedp = nc.declare_dram_parameter("seedp", [128, SWT], I16, isOutput=False)
    outp = nc.declare_dram_parameter("out", [128, NBLK * H], F32, isOutput=True)

    ag_in = [nc.dram_tensor(f"ag_in{l}", [16, NL], F32) for l in (1, 2)]
    ag_out = [nc.dram_tensor(f"ag_out{l}", [128, NL], F32, addr_space="Shared")
              for l in (1, 2)]
    # per-node fp16 alpha_dst rows (own chunk), incl. zero pad wide enough
    # for the last tile's 512-wide adbt broadcast read
    NLP2 = max(NLP, (NT - 1) * NB + 512)
    ad_row = [nc.dram_tensor(f"ad_row{l}", [1, NLP2], F16) for l in (1, 2)]
    # per-edge alpha_dst streams staged by the seed phase
    adeg = [nc.dram_tensor(f"adeg{l}", [128, MWT], F16) for l in (1, 2)]

    with tile.TileContext(nc, num_cores=NC) as tc:
        with tc.tile_pool(name="const", bufs=1) as cpool:
            w1t = cpool.tile([128, KB, H], BF16)
            nc.gpsimd.dma_start(out=w1t[:], in_=w1[:].rearrange("(b p) h -> p b h", p=128))
            # w2 / identity copies aligned to the partitions where out1T /
            # out2T live (matmul needs lhsT and rhs on the same partitions).
            w2t = cpool.tile([48, H], BF16)
            nc.gpsimd.dma_start(out=w2t[32:48, :], in_=w2[:])
            a1t = cpool.tile([16, 16], BF16)
            nc.gpsimd.dma_start(out=a1t[:], in_=a1rep[:])
            ad1t = cpool.tile([16, 16], BF16)
            nc.gpsimd.dma_start(out=ad1t[:], in_=ad1rep[:])
            a2t = cpool.tile([16, 16], BF16)
            nc.gpsimd.dma_start(out=a2t[:], in_=a2rep[:])
            ad2t = cpool.tile([16, 16], BF16)
            nc.gpsimd.dma_start(out=ad2t[:], in_=ad2rep[:])
            b1t = cpool.tile([16, 1], F32)
            nc.sync.dma_start(out=b1t[:], in_=b1p[:])
            b2t = cpool.tile([16, 1], F32)
            nc.sync.dma_start(out=b2t[:], in_=b2p[:])
            selt = cpool.tile([128, 16], F16)
            nc.gpsimd.dma_start(out=selt[:], in_=selp[:])
            idt = cpool.tile([80, 16], F32)
            nc.sync.dma_start(out=idt[64:80, :], in_=identp[:])

            # Stacked per-layer state, one tile so 16-partition rows share
            # column space; engine accesses must start at partition 0/32/64/96:
            # p0-15 wself, p32-47 out1T (bf16, matmul rhs -> base 32),
            # p64-79 out2T (transpose input -> base 64), p96-111 pself.
            stk = cpool.tile([128, NL2], F32)
            wselfv = stk[0:16, 0:NL]
            pselfv = stk[96:112, 0:NL]
            out1v = stk[32:48, 0:W2].bitcast(BF16)      # [16, NL2] bf16
            out2v = stk[64:80, 0:NL2]
            nc.vector.memset(stk[64:80, NL:], 0.0)

            zero16 = cpool.tile([1, NLP2 - NL], F16)
            nc.vector.memset(zero16[:], 0.0)

            idxs_sb = cpool.tile([128, IWT], I16)
            nc.gpsimd.dma_start(out=idxs_sb[:], in_=idxsp[:])

            # ---------------- phase A (layer 1): tables from x^T ----------
            with (
                tc.tile_pool(name="pa", bufs=2) as pa,
                tc.tile_pool(name="pap", bufs=2, space="PSUM") as pap,
            ):
                for c in range(NCH):
                    sl = slice(c * CH, (c + 1) * CH)
                    xt_t = pa.tile([128, KB, CH], BF16, tag="xt")
                    nc.sync.dma_start(
                        out=xt_t[:],
                        in_=xtw[:, c * KB * CH:(c + 1) * KB * CH]
                        .rearrange("p (b n) -> p b n", b=KB))
                    ph = pap.tile([16, CH], F32, tag="ph")
                    for b in range(KB):
                        nc.tensor.matmul(ph[:], lhsT=w1t[:, b, :], rhs=xt_t[:, b, :],
                                         start=(b == 0), stop=(b == KB - 1))
                    hch = pa.tile([16, CH], BF16, tag="hch")
                    nc.scalar.copy(hch[:], ph[:])
                    pas = pap.tile([16, CH], F32, tag="pas")
                    nc.tensor.matmul(pas[:], lhsT=a1t[:], rhs=hch[:], start=True, stop=True)
                    pad_ = pap.tile([16, CH], F32, tag="pad")
                    nc.tensor.matmul(pad_[:], lhsT=ad1t[:], rhs=hch[:], start=True, stop=True)
                    packed = pa.tile([16, CH], F32, tag="packed")
                    pk = packed[:].bitcast(F16)
                    nc.vector.tensor_copy(pk[:, 0::2], hch[:])
                    nc.vector.tensor_copy(pk[:, 1::2], pas[:])
                    nc.sync.dma_start(out=ag_in[0][:, sl], in_=packed[:])
                    adrow = pa.tile([1, CH], F16, tag="adrow")
                    nc.vector.tensor_copy(adrow[:], pad_[0:1, :])
                    nc.sync.dma_start(out=ad_row[0][0:1, sl], in_=adrow[:])
                    # analytic self-loop contribution for this chunk
                    adfull = pa.tile([16, CH], F32, tag="adfull")
                    nc.scalar.copy(adfull[:], pad_[:])
                    tself = pa.tile([16, CH], F32, tag="tself")
                    nc.vector.tensor_add(tself[:], pas[:], adfull[:])
                    nc.vector.scalar_tensor_tensor(
                        tself[:], tself[:], NEG_SLOPE, tself[:],
                        op0=mybir.AluOpType.mult, op1=mybir.AluOpType.max)
                    pselfc = pa.tile([16, CH], F32, tag="pselfc")
                    nc.scalar.activation(pselfc[:], tself[:],
                                         mybir.ActivationFunctionType.Exp)
                    nc.scalar.copy(pselfv[:, sl], pselfc[:])
                    nc.vector.tensor_mul(wselfv[:, sl], pselfc[:], hch[:])

            def run_layer(l, writer):
                nc.sync.dma_start(out=ad_row[l][0:1, NL:], in_=zero16[:])
                nc.gpsimd.collective_compute(
                    "AllGather", mybir.AluOpType.bypass, replica_groups=rg,
                    ins=[ag_in[l][:]], outs=[ag_out[l][:]])
                # ---- seed phase: alpha_dst -> run-start seeds -> scan ----
                with tc.tile_pool(name=f"sd{l}", bufs=3) as sd:
                    for t in range(NT):
                        Tt = T_list[t]
                        NHt, W0t = _halves(Tt)
                        adbt = sd.tile([128, 512], F16, tag="adbt")
                        nc.sync.dma_start(
                            out=adbt[:],
                            in_=ad_row[l][0:1, t * NB:t * NB + 512]
                            .to_broadcast([128, 512]))
                        sidx = sd.tile([128, 512 * (T_max // LS_MAX + 1)],
                                       I16, tag="sidx")
                        nc.sync.dma_start(
                            out=sidx[:, :NHt * 512],
                            in_=seedp[:, seed_off[t]:seed_off[t] + NHt * 512])
                        seeds = sd.tile([128, T_max], F16, tag="seeds")
                        for h in range(NHt):
                            w0, w1_ = h * W0t, min((h + 1) * W0t, Tt)
                            nc.gpsimd.local_scatter(
                                seeds[:, w0:w1_], adbt[:],
                                sidx[:, h * 512:(h + 1) * 512],
                                channels=128, num_elems=w1_ - w0, num_idxs=512)
                        msk = sd.tile([128, T_max], F16, tag="msk")
                        nc.sync.dma_start(
                            out=msk[:, :Tt],
                            in_=maskp[:, mask_off[t]:mask_off[t] + Tt])
                        ade = sd.tile([128, T_max], F16, tag="ade")
                        nc.vector.tensor_tensor_scan(
                            ade[:, :Tt], msk[:, :Tt], seeds[:, :Tt], 0.0,
                            mybir.AluOpType.mult, mybir.AluOpType.add)
                        nc.sync.dma_start(
                            out=adeg[l][:, mask_off[t]:mask_off[t] + Tt],
                            in_=ade[:, :Tt])
                tc.no_sync_barrier()
                # ---- edge phase: h|as gather + per-edge math + seg-sums ----
                with tc.tile_pool(name=f"tab{l}", bufs=1) as tabp:
                    table = tabp.tile([128, NLP], F32, tag="table")
                    nc.sync.dma_start(out=table[:, :NL], in_=ag_out[l][:])
                    nc.vector.memset(table[:, NL:], SENT_PACKED)
                    with (
                        tc.tile_pool(name=f"ed{l}", bufs=2) as ed,
                        tc.tile_pool(name=f"eb{l}", bufs=3) as eb,
                        tc.tile_pool(name=f"e1{l}", bufs=1) as e1,
                        tc.tile_pool(name=f"edp{l}", bufs=2, space="PSUM") as edp,
                    ):
                        # software-pipelined: issue tile t's h|as gather one
                        # iteration ahead of its per-edge math, so the DVE
                        # chain of tile t hides under the gather of t+1 and
                        # GPSIMD stays busy back-to-back.
                        pend = {}

                        def issue(t):
                            Tt = T_list[t]
                            o = idx_off[t]
                            ghs = ed.tile([128, T_max], F32, tag="ghs")
                            nc.gpsimd.ap_gather(
                                ghs[:, :Tt], table[:],
                                idxs_sb[:, o:o + Tt // 16],
                                channels=128, num_elems=NLP, d=1, num_idxs=Tt)
                            ade = e1.tile([128, T_max], F16, tag="adg", bufs=3)
                            nc.sync.dma_start(
                                out=ade[:, :Tt],
                                in_=adeg[l][:, mask_off[t]:mask_off[t] + Tt])
                            msk = e1.tile([128, T_max], F16, tag="mask", bufs=3)
                            nc.sync.dma_start(
                                out=msk[:, :Tt],
                                in_=maskp[:, mask_off[t]:mask_off[t] + Tt])
                            pend[t] = (ghs, ade, msk)

                        def process(t):
                            ghs, ade, msk = pend.pop(t)
                            Tt = T_list[t]
                            o = idx_off[t]
                            i_bnd = idxs_sb[:, o + Tt // 16:o + Tt // 16 + 32]
                            g16 = ghs[:, :Tt].bitcast(F16)   # [128, 2*Tt]
                            tt = ed.tile([128, T_max], F16, tag="tt")
                            nc.vector.tensor_add(tt[:, :Tt], g16[:, 1::2], ade[:, :Tt])
                            nc.vector.scalar_tensor_tensor(
                                tt[:, :Tt], tt[:, :Tt], NEG_SLOPE, tt[:, :Tt],
                                op0=mybir.AluOpType.mult, op1=mybir.AluOpType.max)
                            pp = e1.tile([128, T_max], F16, tag="pp", bufs=2)
                            nc.scalar.activation(pp[:, :Tt], tt[:, :Tt],
                                                 mybir.ActivationFunctionType.Exp)
                            ww = ed.tile([128, T_max], F16, tag="ww")
                            nc.vector.tensor_mul(ww[:, :Tt], pp[:, :Tt], g16[:, 0::2])
                            sc = eb.tile([128, T_max], F32, tag="sc", bufs=2)
                            sc16 = sc[:, :Tt].bitcast(F16)   # (w | p) lanes
                            nc.vector.tensor_tensor_scan(
                                sc16[:, 0::2], msk[:, :Tt], ww[:, :Tt], 0.0,
                                mybir.AluOpType.mult, mybir.AluOpType.add)
                            nc.vector.tensor_tensor_scan(
                                sc16[:, 1::2], msk[:, :Tt], pp[:, :Tt], 0.0,
                                mybir.AluOpType.mult, mybir.AluOpType.add)
                            bg = e1.tile([128, 512], F32, tag="bg", bufs=2)
                            nc.gpsimd.ap_gather(
                                bg[:], sc[:, :Tt], i_bnd,
                                channels=128, num_elems=Tt, d=1, num_idxs=512)
                            bg16 = bg[:].bitcast(F16)
                            pu = edp.tile([16, 512], F32, tag="pu")
                            nc.tensor.matmul(pu[:], lhsT=selt[:], rhs=bg16[:, 0::2],
                                             start=True, stop=True)
                            ps = edp.tile([16, 512], F32, tag="ps")
                            nc.tensor.matmul(ps[:], lhsT=selt[:], rhs=bg16[:, 1::2],
                                             start=True, stop=True)
                            # add analytic self-loop terms, then normalize
                            den = ed.tile([16, NB], F32, tag="s16", bufs=4)
                            nc.vector.tensor_add(den[:], ps[:, :NB],
                                                 pselfv[:, t * NB:(t + 1) * NB])
                            rc = ed.tile([16, NB], F32, tag="s16", bufs=4)
                            nc.vector.reciprocal_approx_fast(out=rc[:], in_=den[:])
                            num = ed.tile([16, NB], F32, tag="s16", bufs=4)
                            nc.vector.tensor_add(num[:], pu[:, :NB],
                                                 wselfv[:, t * NB:(t + 1) * NB])
                            uv = ed.tile([16, NB], F32, tag="s16", bufs=4)
                            nc.vector.tensor_mul(uv[:], num[:], rc[:])
                            writer(t, uv)

                        issue(0)
                        for t in range(1, NT):
                            issue(t)
                            process(t - 1)
                        process(NT - 1)

            def write1(t, uv):
                nc.scalar.activation(out1v[:, t * NB:(t + 1) * NB], uv[:],
                                     mybir.ActivationFunctionType.Relu,
                                     bias=b1t[:, 0:1])

            def write2(t, uv):
                nc.vector.tensor_scalar_add(out2v[:, t * NB:(t + 1) * NB], uv[:],
                                            b2t[:, 0:1])

            run_layer(0, write1)

            # ---------------- phase A (layer 2): tables from out1T --------
            with (
                tc.tile_pool(name="pa2", bufs=2) as pa2,
                tc.tile_pool(name="pap2", bufs=2, space="PSUM") as pap2,
            ):
                for c in range(NCH):
                    sl = slice(c * CH, (c + 1) * CH)
                    ph = pap2.tile([16, CH], F32, tag="ph2")
                    nc.tensor.matmul(ph[:], lhsT=w2t[32:48, :], rhs=out1v[:, sl],
                                     start=True, stop=True)
                    h2ch = pa2.tile([16, CH], BF16, tag="h2ch")
                    nc.scalar.copy(h2ch[:], ph[:])
                    pas = pap2.tile([16, CH], F32, tag="pas2")
                    nc.tensor.matmul(pas[:], lhsT=a2t[:], rhs=h2ch[:], start=True, stop=True)
                    pad_ = pap2.tile([16, CH], F32, tag="pad2")
                    nc.tensor.matmul(pad_[:], lhsT=ad2t[:], rhs=h2ch[:], start=True, stop=True)
                    packed = pa2.tile([16, CH], F32, tag="packed2")
                    pk = packed[:].bitcast(F16)
                    nc.vector.tensor_copy(pk[:, 0::2], h2ch[:])
                    nc.vector.tensor_copy(pk[:, 1::2], pas[:])
                    nc.sync.dma_start(out=ag_in[1][:, sl], in_=packed[:])
                    adrow = pa2.tile([1, CH], F16, tag="adrow2")
                    nc.vector.tensor_copy(adrow[:], pad_[0:1, :])
                    nc.sync.dma_start(out=ad_row[1][0:1, sl], in_=adrow[:])
                    adfull = pa2.tile([16, CH], F32, tag="adfull2")
                    nc.scalar.copy(adfull[:], pad_[:])
                    tself = pa2.tile([16, CH], F32, tag="tself2")
                    nc.vector.tensor_add(tself[:], pas[:], adfull[:])
                    nc.vector.scalar_tensor_tensor(
                        tself[:], tself[:], NEG_SLOPE, tself[:],
                        op0=mybir.AluOpType.mult, op1=mybir.AluOpType.max)
                    pselfc = pa2.tile([16, CH], F32, tag="pselfc2")
                    nc.scalar.activation(pselfc[:], tself[:],
                                         mybir.ActivationFunctionType.Exp)
                    nc.scalar.copy(pselfv[:, sl], pselfc[:])
                    nc.vector.tensor_mul(wselfv[:, sl], pselfc[:], h2ch[:])

            run_layer(1, write2)

            # ---------------- log_softmax + transpose + store -------------
            with (
                tc.tile_pool(name="fin", bufs=2) as fin,
                tc.tile_pool(name="finp", bufs=2, space="PSUM") as finp,
                tc.tile_pool(name="fino", bufs=1) as fino,
            ):
                nodemaj = fino.tile([128, NBLK, H], F32, tag="nodemaj")
                for r in range(0, NBLK, 16):
                    nb = min(16, NBLK - r)
                    ptp = finp.tile([128, 16, 16], F32, tag="ptp")
                    for j in range(nb):
                        nc.tensor.transpose(ptp[:, j, :],
                                            out2v[:, (r + j) * 128:(r + j + 1) * 128],
                                            idt[64:80, :])
                    nc.vector.tensor_copy(nodemaj[:, r:r + nb, :], ptp[:, 0:nb, :])
                mx = fin.tile([128, NBLK], F32, tag="mx")
                nc.vector.tensor_reduce(mx[:], nodemaj[:], axis=mybir.AxisListType.X,
                                        op=mybir.AluOpType.max)
                zz = fino.tile([128, NBLK, H], F32, tag="zz")
                nc.vector.tensor_sub(zz[:], nodemaj[:],
                                     mx[:, :, None].to_broadcast([128, NBLK, H]))
                es = fino.tile([128, NBLK, H], F32, tag="es")
                nc.scalar.activation(es[:], zz[:], mybir.ActivationFunctionType.Exp)
                sm = fin.tile([128, NBLK], F32, tag="sm")
                nc.vector.tensor_reduce(sm[:], es[:], axis=mybir.AxisListType.X,
                                        op=mybir.AluOpType.add)
                ls = fin.tile([128, NBLK], F32, tag="ls")
                nc.scalar.activation(ls[:], sm[:], mybir.ActivationFunctionType.Ln)
                outf = fino.tile([128, NBLK, H], F32, tag="outf")
                nc.vector.tensor_sub(outf[:], zz[:],
                                     ls[:, :, None].to_broadcast([128, NBLK, H]))
                nc.gpsimd.dma_start(out=outp[:].rearrange("p (b h) -> p b h", h=H),
                                    in_=outf[:])

    nc.compile()
    n_reload = sum(
        1
        for f in nc.m.functions
        for bb in f.blocks
        for ins in bb.instructions
        if type(ins).__name__ == "InstPseudoReloadLibraryIndex"
    )
    assert n_reload <= 8, f"library thrash: {n_reload} reloads"
    if split:
        split_waits(nc, max_waits=max_waits, ctrl_max_waits=ctrl_max_waits)
    return nc


CTRL_TYPES = ("InstDrain", "InstNoOp", "InstHalt", "InstEventSemaphore")


def split_waits(nc, max_waits=2, ctrl_max_waits=1):
    """walrus in this container caps sync-waits per instruction; move excess
    waits onto preceding same-engine NoOps (each carrying one wait)."""
    for f in nc.m.functions:
        for bb in f.blocks:
            new_insts, changed = [], False
            for ins in bb.instructions:
                si = ins.sync_info
                cap = (ctrl_max_waits if type(ins).__name__ in CTRL_TYPES
                       else max_waits)
                if si is not None and si.on_wait is not None and len(si.on_wait) > cap:
                    waits = list(si.on_wait)
                    excess, keep = waits[:-cap] if cap else waits, waits[-cap:] if cap else []
                    for i, w in enumerate(excess):
                        nop = mybir.InstNoOp(name=f"{ins.name}-ws{i}", ins=[], outs=[])
                        nop.engine = ins.engine
                        nop.sync_info = mybir.SyncInfo(on_wait=[w], on_update=[])
                        new_insts.append(nop)
                    si.on_wait = keep
                    changed = True
                new_insts.append(ins)
            if changed:
                bb.instructions = new_insts
    # verify the rewrite stuck (pyo3 lists can copy-on-read)
    for f in nc.m.functions:
        for bb in f.blocks:
            for ins in bb.instructions:
                si = ins.sync_info
                cap = (ctrl_max_waits if type(ins).__name__ in CTRL_TYPES
                       else max_waits)
                assert si is None or si.on_wait is None or len(si.on_wait) <= cap, \
                    f"{ins.name}: {len(si.on_wait)} waits > {cap}"


# ------------------------------------------------------------ input packing

def make_in_maps(inputs, cfg, per_core):
    NC, NL, H, F = cfg["NCORES"], cfg["NL"], cfg["H"], cfg["F"]
    NB, CH, NCH = cfg["NB"], cfg["CH"], cfg["NCH"]
    KB = F // 128
    x = np.asarray(inputs["x"], dtype=np.float32)
    sel = np.zeros((128, 16), dtype=np.float32)
    sel[np.arange(128), np.arange(128) % 16] = 1.0
    shared = {
        "w1": np.ascontiguousarray(np.asarray(inputs["W1"], np.float32)),
        "w2": np.ascontiguousarray(np.asarray(inputs["W2"], np.float32)),
        "a1rep": np.ascontiguousarray(np.repeat(np.asarray(inputs["a_src1"], np.float32)[:, None], 16, 1)),
        "ad1rep": np.ascontiguousarray(np.repeat(np.asarray(inputs["a_dst1"], np.float32)[:, None], 16, 1)),
        "a2rep": np.ascontiguousarray(np.repeat(np.asarray(inputs["a_src2"], np.float32)[:, None], 16, 1)),
        "ad2rep": np.ascontiguousarray(np.repeat(np.asarray(inputs["a_dst2"], np.float32)[:, None], 16, 1)),
        "b1p": np.ascontiguousarray(np.asarray(inputs["b1"], np.float32)[:, None]),
        "b2p": np.ascontiguousarray(np.asarray(inputs["b2"], np.float32)[:, None]),
    }
    shared.update({
        "selp": sel,
        "identp": np.eye(16, dtype=np.float32),
    })
    in_maps = []
    for c in range(NC):
        m = dict(shared)
        xpart = x[c * NL:(c + 1) * NL]                    # [NL, F]
        xtw = (xpart.T.reshape(KB, 128, NCH, CH)
               .transpose(1, 2, 0, 3).reshape(128, NCH * KB * CH))
        m["xtw"] = np.ascontiguousarray(xtw.astype(ml_dtypes.bfloat16))
        m["idxs"] = per_core[c]["idxs"]
        m["maskp"] = per_core[c]["maskp"]
        m["seedp"] = per_core[c]["seedp"]
        in_maps.append(m)
    return in_maps


def unshard_output(results, cfg):
    NC, NL, H = cfg["NCORES"], cfg["NL"], cfg["H"]
    NBLK = math.ceil(NL / 128)
    parts = []
    for c in range(NC):
        a = np.asarray(results[c]["out"]).reshape(128, NBLK, H)
        a = a.transpose(1, 0, 2).reshape(NBLK * 128, H)[:NL]
        parts.append(a)
    return np.concatenate(parts, axis=0)


# ------------------------------------------------------------------- driver

_CACHE = {}


def run_on_hw(inputs, cfg, trace=False, tmpdir=None):
    import os
    import shutil
    from concourse.bass_utils import run_bass_kernel_spmd
    if tmpdir is not None and os.path.isdir(tmpdir):
        shutil.rmtree(tmpdir, ignore_errors=True)
    if tmpdir is not None:
        os.makedirs(tmpdir, exist_ok=True)
    T_list, per_core = host_prep(inputs["edge_index"], cfg)
    key = (cfg["N"], tuple(T_list))
    if key not in _CACHE:
        _CACHE[key] = build_nc(cfg, T_list)
    nc = _CACHE[key]
    in_maps = make_in_maps(inputs, cfg, per_core)
    res = run_bass_kernel_spmd(nc, in_maps, list(range(cfg["NCORES"])),
                               trace=trace, tmpdir=tmpdir)
    out = unshard_output(res.results, cfg)
    return out, res


def kernel(**inputs):
    out, _ = run_on_hw(inputs, FULL_CFG)
    return out.astype(np.float32)


# revision 18
# speedup vs baseline: 1.2018x; 1.2018x over previous
"""GAT (2-layer, heads=1) on 8 Trainium2 NeuronCores.

Strategy (1D node partition):
  - Nodes are split into 8 chunks of NL; core c owns dst-chunk c.
  - Layer tables h^T/alpha_src are computed per-chunk (feature-major via
    PE matmuls on x^T), AllGathered, and kept in SBUF as a per-partition-group
    table: partition group g (16 partitions) holds (h|alpha_src) of chunk g,
    packed two fp16 per fp32 word.
  - Edges are bucketed on the host by (dst-core, src-chunk, dst-block) and
    sorted by dst. Each edge tile covers one block of NB dst nodes for all
    8 src-chunk groups at once; per-edge h|alpha_src gathers run on GPSIMD
    (ap_gather), per-edge math on DVE/ACT, and segment-sums via masked
    tensor_tensor_scan (state = mask*state + w, mask=0 at segment starts)
    + a boundary gather at each node's last edge. Group partials are
    combined with a matmul against a 0/1 selection matrix; softmax
    normalization divides at the node level.
  - Per-edge alpha_dst values are NOT gathered. Instead, for each tile a
    GPSIMD local_scatter places each dst-slot's alpha_dst (fp16) at that
    slot's first-edge stream position (host-known structure; per-partition
    independent indices; ~3.4us per 512-idx scatter vs ~64us for the old
    per-edge ap_gather), and a masked scan broadcasts the value along the
    run. The run-start mask itself is host-precomputed structure, DMA'd
    per tile (HWDGE), freeing GPSIMD entirely.
  - local_scatter and ap_gather live in different GPSIMD IRAM libraries, so
    each layer runs all local_scatters in a "seed phase" (results staged in
    DRAM), then all ap_gathers in the edge phase, with a scheduler fence
    between them -> 2 library reloads per layer instead of per-tile thrash.
  - Slot 0 of every bucket is a sentinel; nodes with no edges in a bucket
    point their boundary index at slot 0 (whose running sum is always 0).
  - Softmax max-subtraction is skipped: attention logits here are O(1), and
    alpha = exp(e)/sum(exp(e)) is shift-invariant.

Host preprocessing only reorders/buckets edge indices and emits 0/1 masks
(structure), never touches float data dependent on device results.
"""

import math

import ml_dtypes
import numpy as np

from concourse import bass, bacc, mybir
import concourse.tile as tile

F32 = mybir.dt.float32
BF16 = mybir.dt.bfloat16
F16 = mybir.dt.float16
I16 = mybir.dt.int16
I32 = mybir.dt.int32

NEG_SLOPE = 0.2
# fp32 word 0xF7500000: low fp16 lane (h) = 0, high fp16 lane
# (alpha_src) = -29952 -> exp(lrelu(...)) == 0, so sentinel edges vanish.
SENT_PACKED = float(
    np.frombuffer(
        (np.uint32(np.float16(-29952.0).view(np.uint16)) << np.uint32(16)).tobytes(),
        np.float32,
    )[0]
)

FULL_CFG = dict(
    NCORES=8, N=100000, F=512, H=16,
    NL=12500, NB=500, NT=25, CH=500, NCH=25,
)

LS_MAX = 2046   # local_scatter num_elems limit (num_elems*32 < 2^16)


def _halves(T_e):
    nh = math.ceil(T_e / LS_MAX)
    w0 = (T_e + nh - 1) // nh
    w0 = (w0 + 1) // 2 * 2
    return nh, w0


# ---------------------------------------------------------------- host prep

def _round_up(x, m):
    return (x + m - 1) // m * m


def host_prep(edge_index, cfg):
    """Bucket and sort edges; build device index streams and masks
    (structure only, no float data).

    Returns (T_e, per_core) where per_core[c] has:
      'idxs'  [128, NT*(T16+32)] i16   (src-local gather idx + boundary idx)
      'maskp' [128, NT*T_e]      f16   (1 inside a run, 0 at run starts)
      'seedp' [128, NT*NH*512]   i16   (run-start positions per dst slot,
                                        split into NH stream halves; -1 = none)
    """
    NC, N, NL = cfg["NCORES"], cfg["N"], cfg["NL"]
    NB, NT = cfg["NB"], cfg["NT"]
    G = 8

    # Self-loops are NOT added here: their contribution is computed
    # analytically at the node level on-device (no gather needed).
    src = np.asarray(edge_index[0], dtype=np.int64)
    dst = np.asarray(edge_index[1], dtype=np.int64)

    core = dst // NL
    grp = src // NL
    order = np.lexsort((src, dst, grp, core))
    src, dst, core, grp = (a[order] for a in (src, dst, core, grp))
    blk = (dst % NL) // NB

    bucket = (core * G + grp) * NT + blk
    nbuck = NC * G * NT
    counts = np.bincount(bucket, minlength=nbuck).reshape(NC, G, NT)
    # +1: slot 0 of every bucket is a sentinel (known-zero running sum).
    # Per-tile stream width: max over (core, group) for that tile index
    # (SPMD: all cores share one program, so width is the cross-core max).
    # width multiple of 32 so per-tile idx blocks stay u32-aligned in SBUF
    # (ap_gather ucode reads the idx stream in 32-bit units)
    T_list = tuple(_round_up(int(counts[:, :, t].max()) + 17, 32)
                   for t in range(NT))
    assert max(T_list) <= 32767
    import os
    if os.environ.get("UNIFORM_T"):
        T_list = (_round_up(max(T_list), int(os.environ["UNIFORM_T"])),) * NT
    T_max = max(T_list)

    starts = np.zeros(nbuck, dtype=np.int64)
    starts[1:] = np.cumsum(counts.reshape(-1))[:-1]
    pos = np.arange(src.size) - starts[bucket] + 1

    is_last = np.ones(src.size, dtype=bool)
    is_last[:-1] = ~((bucket[1:] == bucket[:-1]) & (dst[1:] == dst[:-1]))
    is_first = np.ones(src.size, dtype=bool)
    is_first[1:] = ~((bucket[1:] == bucket[:-1]) & (dst[1:] == dst[:-1]))

    srcl = np.full((NC, G, NT, T_max), NL, dtype=np.int16)
    bnd = np.zeros((NC, G, NT, 512), dtype=np.int16)  # default: sentinel slot 0
    mask = np.ones((NC, G, NT, T_max), dtype=np.float16)
    mask[..., 0] = 0.0
    # seeds per tile half, ragged over tiles; NH_t halves of width W0_t
    seed_blocks = [np.full((NC, G, _halves(T_list[t])[0], 512), -1, np.int16)
                   for t in range(NT)]

    c_, g_, b_ = core, grp, blk
    dloc = (dst % NL) % NB
    srcl[c_, g_, b_, pos] = (src % NL).astype(np.int16)
    bnd[c_[is_last], g_[is_last], b_[is_last],
        dloc[is_last]] = pos[is_last].astype(np.int16)
    mask[c_[is_first], g_[is_first], b_[is_first], pos[is_first]] = 0.0
    for t in range(NT):
        selt = is_first & (blk == t)
        p1 = pos[selt]
        W0t = _halves(T_list[t])[1]
        hh = (p1 // W0t).astype(np.int64)
        seed_blocks[t][c_[selt], g_[selt], hh,
                       dloc[selt]] = (p1 - hh * W0t).astype(np.int16)

    def wrap16(a):
        # [NC, G, w] -> [NC, 128, w//16]; w16[c, 16g+p, s] = a[c, g, s*16+p]
        n = a.shape[-1]
        return (a.reshape(NC, G, n // 16, 16)
                 .transpose(0, 1, 3, 2)
                 .reshape(NC, 128, n // 16))

    idx_parts, mask_parts, seed_parts = [], [], []
    for t in range(NT):
        Tt = T_list[t]
        idx_parts.append(wrap16(srcl[:, :, t, :Tt]))
        idx_parts.append(wrap16(bnd[:, :, t, :]))
        mask_parts.append(np.repeat(mask[:, :, t, :Tt], 16, axis=1))
        sb = seed_blocks[t]
        seed_parts.append(np.repeat(
            sb.reshape(NC, G, -1), 16, axis=1))
    idxs = np.ascontiguousarray(np.concatenate(idx_parts, axis=2))
    maskw = np.ascontiguousarray(np.concatenate(mask_parts, axis=2))
    seedw = np.ascontiguousarray(np.concatenate(seed_parts, axis=2))

    per_core = [{"idxs": idxs[c], "maskp": maskw[c], "seedp": seedw[c]}
                for c in range(NC)]
    return T_list, per_core


# ------------------------------------------------------------- device build

def build_nc(cfg, T_list, max_waits=2, ctrl_max_waits=1, split=True):
    NC, N, F, H = cfg["NCORES"], cfg["N"], cfg["F"], cfg["H"]
    NL, NB, NT = cfg["NL"], cfg["NB"], cfg["NT"]
    CH, NCH = cfg["CH"], cfg["NCH"]
    KB = F // 128
    NLP = NL + 16               # table width incl. sentinel
    T_max = max(T_list)
    # per-tile offsets into the concatenated idx / mask / seed streams
    idx_off, mask_off, seed_off = [], [], []
    io = mo = so = 0
    for t in range(NT):
        Tt = T_list[t]
        idx_off.append(io); mask_off.append(mo); seed_off.append(so)
        io += Tt // 16 + 32
        mo += Tt
        so += _halves(Tt)[0] * 512
    IWT, MWT, SWT = io, mo, so
    NBLK = math.ceil(NL / 128)
    NL2 = NBLK * 128
    W2 = NL2 // 2               # fp32 words backing the bf16 out1T row
    rg = [list(range(NC))]

    nc = bacc.Bacc("TRN2", target_bir_lowering=False)

    xtw = nc.declare_dram_parameter("xtw", [128, NCH * KB * CH], BF16, isOutput=False)
    w1 = nc.declare_dram_parameter("w1", [F, H], F32, isOutput=False)
    w2 = nc.declare_dram_parameter("w2", [H, H], F32, isOutput=False)
    a1rep = nc.declare_dram_parameter("a1rep", [H, 16], F32, isOutput=False)
    ad1rep = nc.declare_dram_parameter("ad1rep", [H, 16], F32, isOutput=False)
    a2rep = nc.declare_dram_parameter("a2rep", [H, 16], F32, isOutput=False)
    ad2rep = nc.declare_dram_parameter("ad2rep", [H, 16], F32, isOutput=False)
    b1p = nc.declare_dram_parameter("b1p", [H, 1], F32, isOutput=False)
    b2p = nc.declare_dram_parameter("b2p", [H, 1], F32, isOutput=False)
    selp = nc.declare_dram_parameter("selp", [128, 16], F32, isOutput=False)
    identp = nc.declare_dram_parameter("identp", [16, 16], F32, isOutput=False)
    idxsp = nc.declare_dram_parameter("idxs", [128, IWT], I16, isOutput=False)
    maskp = nc.declare_dram_parameter("maskp", [128, MWT], F16, isOutput=False)
    seedp = nc.declare_dram_parameter("seedp", [128, SWT], I16, isOutput=False)
    outp = nc.declare_dram_parameter("out", [128, NBLK * H], F32, isOutput=True)

    ag_in = [nc.dram_tensor(f"ag_in{l}", [16, NL], F32) for l in (1, 2)]
    ag_out = [nc.dram_tensor(f"ag_out{l}", [128, NL], F32, addr_space="Shared")
              for l in (1, 2)]
    # per-node fp16 alpha_dst rows (own chunk), incl. zero pad wide enough
    # for the last tile's 512-wide adbt broadcast read
    NLP2 = max(NLP, (NT - 1) * NB + 512)
    ad_row = [nc.dram_tensor(f"ad_row{l}", [1, NLP2], F16) for l in (1, 2)]
    # per-edge alpha_dst streams staged by the seed phase
    adeg = [nc.dram_tensor(f"adeg{l}", [128, MWT], F16) for l in (1, 2)]

    with tile.TileContext(nc, num_cores=NC) as tc:
        with tc.tile_pool(name="const", bufs=1) as cpool:
            w1t = cpool.tile([128, KB, H], BF16)
            nc.gpsimd.dma_start(out=w1t[:], in_=w1[:].rearrange("(b p) h -> p b h", p=128))
            # w2 / identity copies aligned to the partitions where out1T /
            # out2T live (matmul needs lhsT and rhs on the same partitions).
            w2t = cpool.tile([48, H], BF16)
            nc.gpsimd.dma_start(out=w2t[32:48, :], in_=w2[:])
            a1t = cpool.tile([16, 16], BF16)
            nc.gpsimd.dma_start(out=a1t[:], in_=a1rep[:])
            ad1t = cpool.tile([16, 16], BF16)
            nc.gpsimd.dma_start(out=ad1t[:], in_=ad1rep[:])
            a2t = cpool.tile([16, 16], BF16)
            nc.gpsimd.dma_start(out=a2t[:], in_=a2rep[:])
            ad2t = cpool.tile([16, 16], BF16)
            nc.gpsimd.dma_start(out=ad2t[:], in_=ad2rep[:])
            b1t = cpool.tile([16, 1], F32)
            nc.sync.dma_start(out=b1t[:], in_=b1p[:])
            b2t = cpool.tile([16, 1], F32)
            nc.sync.dma_start(out=b2t[:], in_=b2p[:])
            selt = cpool.tile([128, 16], F16)
            nc.gpsimd.dma_start(out=selt[:], in_=selp[:])
            idt = cpool.tile([80, 16], F32)
            nc.sync.dma_start(out=idt[64:80, :], in_=identp[:])

            # Stacked per-layer state, one tile so 16-partition rows share
            # column space; engine accesses must start at partition 0/32/64/96:
            # p0-15 wself, p32-47 out1T (bf16, matmul rhs -> base 32),
            # p64-79 out2T (transpose input -> base 64), p96-111 pself.
            stk = cpool.tile([128, NL2], F32)
            wselfv = stk[0:16, 0:NL]
            pselfv = stk[96:112, 0:NL]
            out1v = stk[32:48, 0:W2].bitcast(BF16)      # [16, NL2] bf16
            out2v = stk[64:80, 0:NL2]
            nc.vector.memset(stk[64:80, NL:], 0.0)

            zero16 = cpool.tile([1, NLP2 - NL], F16)
            nc.vector.memset(zero16[:], 0.0)

            idxs_sb = cpool.tile([128, IWT], I16)
            nc.gpsimd.dma_start(out=idxs_sb[:], in_=idxsp[:])

            # ---------------- phase A (layer 1): tables from x^T ----------
            with (
                tc.tile_pool(name="pa", bufs=2) as pa,
                tc.tile_pool(name="pap", bufs=2, space="PSUM") as pap,
            ):
                for c in range(NCH):
                    sl = slice(c * CH, (c + 1) * CH)
                    xt_t = pa.tile([128, KB, CH], BF16, tag="xt")
                    nc.sync.dma_start(
                        out=xt_t[:],
                        in_=xtw[:, c * KB * CH:(c + 1) * KB * CH]
                        .rearrange("p (b n) -> p b n", b=KB))
                    ph = pap.tile([16, CH], F32, tag="ph")
                    for b in range(KB):
                        nc.tensor.matmul(ph[:], lhsT=w1t[:, b, :], rhs=xt_t[:, b, :],
                                         start=(b == 0), stop=(b == KB - 1))
                    hch = pa.tile([16, CH], BF16, tag="hch")
                    nc.scalar.copy(hch[:], ph[:])
                    pas = pap.tile([16, CH], F32, tag="pas")
                    nc.tensor.matmul(pas[:], lhsT=a1t[:], rhs=hch[:], start=True, stop=True)
                    pad_ = pap.tile([16, CH], F32, tag="pad")
                    nc.tensor.matmul(pad_[:], lhsT=ad1t[:], rhs=hch[:], start=True, stop=True)
                    packed = pa.tile([16, CH], F32, tag="packed")
                    pk = packed[:].bitcast(F16)
                    nc.vector.tensor_copy(pk[:, 0::2], hch[:])
                    nc.vector.tensor_copy(pk[:, 1::2], pas[:])
                    nc.sync.dma_start(out=ag_in[0][:, sl], in_=packed[:])
                    adrow = pa.tile([1, CH], F16, tag="adrow")
                    nc.vector.tensor_copy(adrow[:], pad_[0:1, :])
                    nc.sync.dma_start(out=ad_row[0][0:1, sl], in_=adrow[:])
                    # analytic self-loop contribution for this chunk
                    adfull = pa.tile([16, CH], F32, tag="adfull")
                    nc.scalar.copy(adfull[:], pad_[:])
                    tself = pa.tile([16, CH], F32, tag="tself")
                    nc.vector.tensor_add(tself[:], pas[:], adfull[:])
                    nc.vector.scalar_tensor_tensor(
                        tself[:], tself[:], NEG_SLOPE, tself[:],
                        op0=mybir.AluOpType.mult, op1=mybir.AluOpType.max)
                    pselfc = pa.tile([16, CH], F32, tag="pselfc")
                    nc.scalar.activation(pselfc[:], tself[:],
                                         mybir.ActivationFunctionType.Exp)
                    nc.scalar.copy(pselfv[:, sl], pselfc[:])
                    nc.vector.tensor_mul(wselfv[:, sl], pselfc[:], hch[:])

            def run_layer(l, writer):
                nc.sync.dma_start(out=ad_row[l][0:1, NL:], in_=zero16[:])
                nc.gpsimd.collective_compute(
                    "AllGather", mybir.AluOpType.bypass, replica_groups=rg,
                    ins=[ag_in[l][:]], outs=[ag_out[l][:]])
                # ---- seed phase: alpha_dst -> run-start seeds -> scan ----
                with tc.tile_pool(name=f"sd{l}", bufs=3) as sd:
                    for t in range(NT):
                        Tt = T_list[t]
                        NHt, W0t = _halves(Tt)
                        adbt = sd.tile([128, 512], F16, tag="adbt")
                        nc.sync.dma_start(
                            out=adbt[:],
                            in_=ad_row[l][0:1, t * NB:t * NB + 512]
                            .to_broadcast([128, 512]))
                        sidx = sd.tile([128, 512 * (T_max // LS_MAX + 1)],
                                       I16, tag="sidx")
                        nc.sync.dma_start(
                            out=sidx[:, :NHt * 512],
                            in_=seedp[:, seed_off[t]:seed_off[t] + NHt * 512])
                        seeds = sd.tile([128, T_max], F16, tag="seeds")
                        for h in range(NHt):
                            w0, w1_ = h * W0t, min((h + 1) * W0t, Tt)
                            nc.gpsimd.local_scatter(
                                seeds[:, w0:w1_], adbt[:],
                                sidx[:, h * 512:(h + 1) * 512],
                                channels=128, num_elems=w1_ - w0, num_idxs=512)
                        msk = sd.tile([128, T_max], F16, tag="msk")
                        nc.sync.dma_start(
                            out=msk[:, :Tt],
                            in_=maskp[:, mask_off[t]:mask_off[t] + Tt])
                        ade = sd.tile([128, T_max], F16, tag="ade")
                        nc.vector.tensor_tensor_scan(
                            ade[:, :Tt], msk[:, :Tt], seeds[:, :Tt], 0.0,
                            mybir.AluOpType.mult, mybir.AluOpType.add)
                        nc.sync.dma_start(
                            out=adeg[l][:, mask_off[t]:mask_off[t] + Tt],
                            in_=ade[:, :Tt])
                tc.no_sync_barrier()
                # ---- edge phase: h|as gather + per-edge math + seg-sums ----
                with tc.tile_pool(name=f"tab{l}", bufs=1) as tabp:
                    table = tabp.tile([128, NLP], F32, tag="table")
                    nc.sync.dma_start(out=table[:, :NL], in_=ag_out[l][:])
                    nc.vector.memset(table[:, NL:], SENT_PACKED)
                    with (
                        tc.tile_pool(name=f"ed{l}", bufs=2) as ed,
                        tc.tile_pool(name=f"eb{l}", bufs=3) as eb,
                        tc.tile_pool(name=f"e1{l}", bufs=1) as e1,
                        tc.tile_pool(name=f"edp{l}", bufs=2, space="PSUM") as edp,
                    ):
                        # software-pipelined: issue tile t's h|as gather one
                        # iteration ahead of its per-edge math, so the DVE
                        # chain of tile t hides under the gather of t+1 and
                        # GPSIMD stays busy back-to-back.
                        pend = {}

                        def issue(t):
                            Tt = T_list[t]
                            o = idx_off[t]
                            ghs = ed.tile([128, T_max], F32, tag="ghs")
                            nc.gpsimd.ap_gather(
                                ghs[:, :Tt], table[:],
                                idxs_sb[:, o:o + Tt // 16],
                                channels=128, num_elems=NLP, d=1, num_idxs=Tt)
                            ade = e1.tile([128, T_max], F16, tag="adg", bufs=3)
                            nc.sync.dma_start(
                                out=ade[:, :Tt],
                                in_=adeg[l][:, mask_off[t]:mask_off[t] + Tt])
                            msk = e1.tile([128, T_max], F16, tag="mask", bufs=3)
                            nc.sync.dma_start(
                                out=msk[:, :Tt],
                                in_=maskp[:, mask_off[t]:mask_off[t] + Tt])
                            pend[t] = (ghs, ade, msk)

                        def process(t):
                            ghs, ade, msk = pend.pop(t)
                            Tt = T_list[t]
                            o = idx_off[t]
                            i_bnd = idxs_sb[:, o + Tt // 16:o + Tt // 16 + 32]
                            g16 = ghs[:, :Tt].bitcast(F16)   # [128, 2*Tt]
                            tt = ed.tile([128, T_max], F16, tag="tt")
                            nc.vector.tensor_add(tt[:, :Tt], g16[:, 1::2], ade[:, :Tt])
                            nc.vector.scalar_tensor_tensor(
                                tt[:, :Tt], tt[:, :Tt], NEG_SLOPE, tt[:, :Tt],
                                op0=mybir.AluOpType.mult, op1=mybir.AluOpType.max)
                            pp = e1.tile([128, T_max], F16, tag="pp", bufs=2)
                            nc.scalar.activation(pp[:, :Tt], tt[:, :Tt],
                                                 mybir.ActivationFunctionType.Exp)
                            ww = ed.tile([128, T_max], F16, tag="ww")
                            nc.vector.tensor_mul(ww[:, :Tt], pp[:, :Tt], g16[:, 0::2])
                            sc = eb.tile([128, T_max], F32, tag="sc", bufs=2)
                            sc16 = sc[:, :Tt].bitcast(F16)   # (w | p) lanes
                            nc.vector.tensor_tensor_scan(
                                sc16[:, 0::2], msk[:, :Tt], ww[:, :Tt], 0.0,
                                mybir.AluOpType.mult, mybir.AluOpType.add)
                            nc.vector.tensor_tensor_scan(
                                sc16[:, 1::2], msk[:, :Tt], pp[:, :Tt], 0.0,
                                mybir.AluOpType.mult, mybir.AluOpType.add)
                            bg = e1.tile([128, 512], F32, tag="bg", bufs=2)
                            nc.gpsimd.ap_gather(
                                bg[:], sc[:, :Tt], i_bnd,
                                channels=128, num_elems=Tt, d=1, num_idxs=512)
                            bg16 = bg[:].bitcast(F16)
                            pu = edp.tile([16, 512], F32, tag="pu")
                            nc.tensor.matmul(pu[:], lhsT=selt[:], rhs=bg16[:, 0::2],
                                             start=True, stop=True)
                            ps = edp.tile([16, 512], F32, tag="ps")
                            nc.tensor.matmul(ps[:], lhsT=selt[:], rhs=bg16[:, 1::2],
                                             start=True, stop=True)
                            # add analytic self-loop terms, then normalize
                            den = ed.tile([16, NB], F32, tag="s16", bufs=4)
                            nc.vector.tensor_add(den[:], ps[:, :NB],
                                                 pselfv[:, t * NB:(t + 1) * NB])
                            rc = ed.tile([16, NB], F32, tag="s16", bufs=4)
                            nc.vector.reciprocal_approx_fast(out=rc[:], in_=den[:])
                            num = ed.tile([16, NB], F32, tag="s16", bufs=4)
                            nc.vector.tensor_add(num[:], pu[:, :NB],
                                                 wselfv[:, t * NB:(t + 1) * NB])
                            uv = ed.tile([16, NB], F32, tag="s16", bufs=4)
                            nc.vector.tensor_mul(uv[:], num[:], rc[:])
                            writer(t, uv)

                        issue(0)
                        for t in range(1, NT):
                            issue(t)
                            process(t - 1)
                        process(NT - 1)

            def write1(t, uv):
                nc.scalar.activation(out1v[:, t * NB:(t + 1) * NB], uv[:],
                                     mybir.ActivationFunctionType.Relu,
                                     bias=b1t[:, 0:1])

            def write2(t, uv):
                nc.vector.tensor_scalar_add(out2v[:, t * NB:(t + 1) * NB], uv[:],
                                            b2t[:, 0:1])

            run_layer(0, write1)

            # ---------------- phase A (layer 2): tables from out1T --------
            with (
                tc.tile_pool(name="pa2", bufs=2) as pa2,
                tc.tile_pool(name="pap2", bufs=2, space="PSUM") as pap2,
            ):
                for c in range(NCH):
                    sl = slice(c * CH, (c + 1) * CH)
                    ph = pap2.tile([16, CH], F32, tag="ph2")
                    nc.tensor.matmul(ph[:], lhsT=w2t[32:48, :], rhs=out1v[:, sl],
                                     start=True, stop=True)
                    h2ch = pa2.tile([16, CH], BF16, tag="h2ch")
                    nc.scalar.copy(h2ch[:], ph[:])
                    pas = pap2.tile([16, CH], F32, tag="pas2")
                    nc.tensor.matmul(pas[:], lhsT=a2t[:], rhs=h2ch[:], start=True, stop=True)
                    pad_ = pap2.tile([16, CH], F32, tag="pad2")
                    nc.tensor.matmul(pad_[:], lhsT=ad2t[:], rhs=h2ch[:], start=True, stop=True)
                    packed = pa2.tile([16, CH], F32, tag="packed2")
                    pk = packed[:].bitcast(F16)
                    nc.vector.tensor_copy(pk[:, 0::2], h2ch[:])
                    nc.vector.tensor_copy(pk[:, 1::2], pas[:])
                    nc.sync.dma_start(out=ag_in[1][:, sl], in_=packed[:])
                    adrow = pa2.tile([1, CH], F16, tag="adrow2")
                    nc.vector.tensor_copy(adrow[:], pad_[0:1, :])
                    nc.sync.dma_start(out=ad_row[1][0:1, sl], in_=adrow[:])
                    adfull = pa2.tile([16, CH], F32, tag="adfull2")
                    nc.scalar.copy(adfull[:], pad_[:])
                    tself = pa2.tile([16, CH], F32, tag="tself2")
                    nc.vector.tensor_add(tself[:], pas[:], adfull[:])
                    nc.vector.scalar_tensor_tensor(
                        tself[:], tself[:], NEG_SLOPE, tself[:],
                        op0=mybir.AluOpType.mult, op1=mybir.AluOpType.max)
                    pselfc = pa2.tile([16, CH], F32, tag="pselfc2")
                    nc.scalar.activation(pselfc[:], tself[:],
                                         mybir.ActivationFunctionType.Exp)
                    nc.scalar.copy(pselfv[:, sl], pselfc[:])
                    nc.vector.tensor_mul(wselfv[:, sl], pselfc[:], h2ch[:])

            run_layer(1, write2)

            # ---------------- log_softmax + transpose + store -------------
            with (
                tc.tile_pool(name="fin", bufs=2) as fin,
                tc.tile_pool(name="finp", bufs=2, space="PSUM") as finp,
                tc.tile_pool(name="fino", bufs=1) as fino,
            ):
                nodemaj = fino.tile([128, NBLK, H], F32, tag="nodemaj")
                for r in range(0, NBLK, 16):
                    nb = min(16, NBLK - r)
                    ptp = finp.tile([128, 16, 16], F32, tag="ptp")
                    for j in range(nb):
                        nc.tensor.transpose(ptp[:, j, :],
                                            out2v[:, (r + j) * 128:(r + j + 1) * 128],
                                            idt[64:80, :])
                    nc.vector.tensor_copy(nodemaj[:, r:r + nb, :], ptp[:, 0:nb, :])
                mx = fin.tile([128, NBLK], F32, tag="mx")
                nc.vector.tensor_reduce(mx[:], nodemaj[:], axis=mybir.AxisListType.X,
                                        op=mybir.AluOpType.max)
                zz = fino.tile([128, NBLK, H], F32, tag="zz")
                nc.vector.tensor_sub(zz[:], nodemaj[:],
                                     mx[:, :, None].to_broadcast([128, NBLK, H]))
                es = fino.tile([128, NBLK, H], F32, tag="es")
                nc.scalar.activation(es[:], zz[:], mybir.ActivationFunctionType.Exp)
                sm = fin.tile([128, NBLK], F32, tag="sm")
                nc.vector.tensor_reduce(sm[:], es[:], axis=mybir.AxisListType.X,
                                        op=mybir.AluOpType.add)
                ls = fin.tile([128, NBLK], F32, tag="ls")
                nc.scalar.activation(ls[:], sm[:], mybir.ActivationFunctionType.Ln)
                outf = fino.tile([128, NBLK, H], F32, tag="outf")
                nc.vector.tensor_sub(outf[:], zz[:],
                                     ls[:, :, None].to_broadcast([128, NBLK, H]))
                nc.gpsimd.dma_start(out=outp[:].rearrange("p (b h) -> p b h", h=H),
                                    in_=outf[:])

    nc.compile()
    n_reload = sum(
        1
        for f in nc.m.functions
        for bb in f.blocks
        for ins in bb.instructions
        if type(ins).__name__ == "InstPseudoReloadLibraryIndex"
    )
    assert n_reload <= 8, f"library thrash: {n_reload} reloads"
    if split:
        split_waits(nc, max_waits=max_waits, ctrl_max_waits=ctrl_max_waits)
    return nc


CTRL_TYPES = ("InstDrain", "InstNoOp", "InstHalt", "InstEventSemaphore")


def split_waits(nc, max_waits=2, ctrl_max_waits=1):
    """walrus in this container caps sync-waits per instruction; move excess
    waits onto preceding same-engine NoOps (each carrying one wait)."""
    for f in nc.m.functions:
        for bb in f.blocks:
            new_insts, changed = [], False
            for ins in bb.instructions:
                si = ins.sync_info
                cap = (ctrl_max_waits if type(ins).__name__ in CTRL_TYPES
                       else max_waits)
                if si is not None and si.on_wait is not None and len(si.on_wait) > cap:
                    waits = list(si.on_wait)
                    excess, keep = waits[:-cap] if cap else waits, waits[-cap:] if cap else []
                    for i, w in enumerate(excess):
                        nop = mybir.InstNoOp(name=f"{ins.name}-ws{i}", ins=[], outs=[])
                        nop.engine = ins.engine
                        nop.sync_info = mybir.SyncInfo(on_wait=[w], on_update=[])
                        new_insts.append(nop)
                    si.on_wait = keep
                    changed = True
                new_insts.append(ins)
            if changed:
                bb.instructions = new_insts
    # verify the rewrite stuck (pyo3 lists can copy-on-read)
    for f in nc.m.functions:
        for bb in f.blocks:
            for ins in bb.instructions:
                si = ins.sync_info
                cap = (ctrl_max_waits if type(ins).__name__ in CTRL_TYPES
                       else max_waits)
                assert si is None or si.on_wait is None or len(si.on_wait) <= cap, \
                    f"{ins.name}: {len(si.on_wait)} waits > {cap}"


# ------------------------------------------------------------ input packing

def make_in_maps(inputs, cfg, per_core):
    NC, NL, H, F = cfg["NCORES"], cfg["NL"], cfg["H"], cfg["F"]
    NB, CH, NCH = cfg["NB"], cfg["CH"], cfg["NCH"]
    KB = F // 128
    x = np.asarray(inputs["x"], dtype=np.float32)
    sel = np.zeros((128, 16), dtype=np.float32)
    sel[np.arange(128), np.arange(128) % 16] = 1.0
    shared = {
        "w1": np.ascontiguousarray(np.asarray(inputs["W1"], np.float32)),
        "w2": np.ascontiguousarray(np.asarray(inputs["W2"], np.float32)),
        "a1rep": np.ascontiguousarray(np.repeat(np.asarray(inputs["a_src1"], np.float32)[:, None], 16, 1)),
        "ad1rep": np.ascontiguousarray(np.repeat(np.asarray(inputs["a_dst1"], np.float32)[:, None], 16, 1)),
        "a2rep": np.ascontiguousarray(np.repeat(np.asarray(inputs["a_src2"], np.float32)[:, None], 16, 1)),
        "ad2rep": np.ascontiguousarray(np.repeat(np.asarray(inputs["a_dst2"], np.float32)[:, None], 16, 1)),
        "b1p": np.ascontiguousarray(np.asarray(inputs["b1"], np.float32)[:, None]),
        "b2p": np.ascontiguousarray(np.asarray(inputs["b2"], np.float32)[:, None]),
    }
    shared.update({
        "selp": sel,
        "identp": np.eye(16, dtype=np.float32),
    })
    in_maps = []
    for c in range(NC):
        m = dict(shared)
        xpart = x[c * NL:(c + 1) * NL]                    # [NL, F]
        xtw = (xpart.T.reshape(KB, 128, NCH, CH)
               .transpose(1, 2, 0, 3).reshape(128, NCH * KB * CH))
        m["xtw"] = np.ascontiguousarray(xtw.astype(ml_dtypes.bfloat16))
        m["idxs"] = per_core[c]["idxs"]
        m["maskp"] = per_core[c]["maskp"]
        m["seedp"] = per_core[c]["seedp"]
        in_maps.append(m)
    return in_maps


def unshard_output(results, cfg):
    NC, NL, H = cfg["NCORES"], cfg["NL"], cfg["H"]
    NBLK = math.ceil(NL / 128)
    parts = []
    for c in range(NC):
        a = np.asarray(results[c]["out"]).reshape(128, NBLK, H)
        a = a.transpose(1, 0, 2).reshape(NBLK * 128, H)[:NL]
        parts.append(a)
    return np.concatenate(parts, axis=0)


# ------------------------------------------------------------------- driver

_CACHE = {}


def run_on_hw(inputs, cfg, trace=False, tmpdir=None):
    import os
    import shutil
    from concourse.bass_utils import run_bass_kernel_spmd
    if tmpdir is not None and os.path.isdir(tmpdir):
        shutil.rmtree(tmpdir, ignore_errors=True)
    if tmpdir is not None:
        os.makedirs(tmpdir, exist_ok=True)
    T_list, per_core = host_prep(inputs["edge_index"], cfg)
    key = (cfg["N"], tuple(T_list))
    if key not in _CACHE:
        _CACHE[key] = build_nc(cfg, T_list)
    nc = _CACHE[key]
    in_maps = make_in_maps(inputs, cfg, per_core)
    res = run_bass_kernel_spmd(nc, in_maps, list(range(cfg["NCORES"])),
                               trace=trace, tmpdir=tmpdir)
    out = unshard_output(res.results, cfg)
    return out, res


def kernel(**inputs):
    out, _ = run_on_hw(inputs, FULL_CFG)
    return out.astype(np.float32)


# revision 19
# speedup vs baseline: 1.2029x; 1.0009x over previous
"""GAT (2-layer, heads=1) on 8 Trainium2 NeuronCores.

Strategy (1D node partition):
  - Nodes are split into 8 chunks of NL; core c owns dst-chunk c.
  - Layer tables h^T/alpha_src are computed per-chunk (feature-major via
    PE matmuls on x^T), AllGathered, and kept in SBUF as a per-partition-group
    table: partition group g (16 partitions) holds (h|alpha_src) of chunk g,
    packed two fp16 per fp32 word.
  - Edges are bucketed on the host by (dst-core, src-chunk, dst-block) and
    sorted by dst. Each edge tile covers one block of NB dst nodes for all
    8 src-chunk groups at once; per-edge h|alpha_src gathers run on GPSIMD
    (ap_gather), per-edge math on DVE/ACT, and segment-sums via masked
    tensor_tensor_scan (state = mask*state + w, mask=0 at segment starts)
    + a boundary gather at each node's last edge. Group partials are
    combined with a matmul against a 0/1 selection matrix; softmax
    normalization divides at the node level.
  - Per-edge alpha_dst values are NOT gathered. Instead, for each tile a
    GPSIMD local_scatter places each dst-slot's alpha_dst (fp16) at that
    slot's first-edge stream position (host-known structure; per-partition
    independent indices; ~3.4us per 512-idx scatter vs ~64us for the old
    per-edge ap_gather), and a masked scan broadcasts the value along the
    run. The run-start mask itself is host-precomputed structure, DMA'd
    per tile (HWDGE), freeing GPSIMD entirely.
  - local_scatter and ap_gather live in different GPSIMD IRAM libraries, so
    each layer runs all local_scatters in a "seed phase" (results staged in
    DRAM), then all ap_gathers in the edge phase, with a scheduler fence
    between them -> 2 library reloads per layer instead of per-tile thrash.
  - Slot 0 of every bucket is a sentinel; nodes with no edges in a bucket
    point their boundary index at slot 0 (whose running sum is always 0).
  - Softmax max-subtraction is skipped: attention logits here are O(1), and
    alpha = exp(e)/sum(exp(e)) is shift-invariant.

Host preprocessing only reorders/buckets edge indices and emits 0/1 masks
(structure), never touches float data dependent on device results.
"""

import math

import ml_dtypes
import numpy as np

from concourse import bass, bacc, mybir
import concourse.tile as tile
from concourse.tile_rust import add_dep_helper

F32 = mybir.dt.float32
BF16 = mybir.dt.bfloat16
F16 = mybir.dt.float16
I16 = mybir.dt.int16
I32 = mybir.dt.int32

NEG_SLOPE = 0.2
# fp32 word 0xF7500000: low fp16 lane (h) = 0, high fp16 lane
# (alpha_src) = -29952 -> exp(lrelu(...)) == 0, so sentinel edges vanish.
SENT_PACKED = float(
    np.frombuffer(
        (np.uint32(np.float16(-29952.0).view(np.uint16)) << np.uint32(16)).tobytes(),
        np.float32,
    )[0]
)

FULL_CFG = dict(
    NCORES=8, N=100000, F=512, H=16,
    NL=12500, NB=500, NT=25, CH=500, NCH=25,
)

LS_MAX = 2046   # local_scatter num_elems limit (num_elems*32 < 2^16)


def _halves(T_e):
    nh = math.ceil(T_e / LS_MAX)
    w0 = (T_e + nh - 1) // nh
    w0 = (w0 + 1) // 2 * 2
    return nh, w0


# ---------------------------------------------------------------- host prep

def _round_up(x, m):
    return (x + m - 1) // m * m


def host_prep(edge_index, cfg):
    """Bucket and sort edges; build device index streams and masks
    (structure only, no float data).

    Returns (T_e, per_core) where per_core[c] has:
      'idxs'  [128, NT*(T16+32)] i16   (src-local gather idx + boundary idx)
      'maskp' [128, NT*T_e]      f16   (1 inside a run, 0 at run starts)
      'seedp' [128, NT*NH*512]   i16   (run-start positions per dst slot,
                                        split into NH stream halves; -1 = none)
    """
    NC, N, NL = cfg["NCORES"], cfg["N"], cfg["NL"]
    NB, NT = cfg["NB"], cfg["NT"]
    G = 8

    # Self-loops are NOT added here: their contribution is computed
    # analytically at the node level on-device (no gather needed).
    src = np.asarray(edge_index[0], dtype=np.int64)
    dst = np.asarray(edge_index[1], dtype=np.int64)

    core = dst // NL
    grp = src // NL
    order = np.lexsort((src, dst, grp, core))
    src, dst, core, grp = (a[order] for a in (src, dst, core, grp))
    blk = (dst % NL) // NB

    bucket = (core * G + grp) * NT + blk
    nbuck = NC * G * NT
    counts = np.bincount(bucket, minlength=nbuck).reshape(NC, G, NT)
    # +1: slot 0 of every bucket is a sentinel (known-zero running sum).
    # Per-tile stream width: max over (core, group) for that tile index
    # (SPMD: all cores share one program, so width is the cross-core max).
    # width multiple of 32 so per-tile idx blocks stay u32-aligned in SBUF
    # (ap_gather ucode reads the idx stream in 32-bit units)
    T_list = tuple(_round_up(int(counts[:, :, t].max()) + 17, 32)
                   for t in range(NT))
    assert max(T_list) <= 32767
    import os
    if os.environ.get("UNIFORM_T"):
        T_list = (_round_up(max(T_list), int(os.environ["UNIFORM_T"])),) * NT
    T_max = max(T_list)

    starts = np.zeros(nbuck, dtype=np.int64)
    starts[1:] = np.cumsum(counts.reshape(-1))[:-1]
    pos = np.arange(src.size) - starts[bucket] + 1

    is_last = np.ones(src.size, dtype=bool)
    is_last[:-1] = ~((bucket[1:] == bucket[:-1]) & (dst[1:] == dst[:-1]))
    is_first = np.ones(src.size, dtype=bool)
    is_first[1:] = ~((bucket[1:] == bucket[:-1]) & (dst[1:] == dst[:-1]))

    srcl = np.full((NC, G, NT, T_max), NL, dtype=np.int16)
    bnd = np.zeros((NC, G, NT, 512), dtype=np.int16)  # default: sentinel slot 0
    mask = np.ones((NC, G, NT, T_max), dtype=np.float16)
    mask[..., 0] = 0.0
    # seeds per tile half, ragged over tiles; NH_t halves of width W0_t
    seed_blocks = [np.full((NC, G, _halves(T_list[t])[0], 512), -1, np.int16)
                   for t in range(NT)]

    c_, g_, b_ = core, grp, blk
    dloc = (dst % NL) % NB
    srcl[c_, g_, b_, pos] = (src % NL).astype(np.int16)
    bnd[c_[is_last], g_[is_last], b_[is_last],
        dloc[is_last]] = pos[is_last].astype(np.int16)
    mask[c_[is_first], g_[is_first], b_[is_first], pos[is_first]] = 0.0
    for t in range(NT):
        selt = is_first & (blk == t)
        p1 = pos[selt]
        W0t = _halves(T_list[t])[1]
        hh = (p1 // W0t).astype(np.int64)
        seed_blocks[t][c_[selt], g_[selt], hh,
                       dloc[selt]] = (p1 - hh * W0t).astype(np.int16)

    def wrap16(a):
        # [NC, G, w] -> [NC, 128, w//16]; w16[c, 16g+p, s] = a[c, g, s*16+p]
        n = a.shape[-1]
        return (a.reshape(NC, G, n // 16, 16)
                 .transpose(0, 1, 3, 2)
                 .reshape(NC, 128, n // 16))

    idx_parts, mask_parts, seed_parts = [], [], []
    for t in range(NT):
        Tt = T_list[t]
        idx_parts.append(wrap16(srcl[:, :, t, :Tt]))
        idx_parts.append(wrap16(bnd[:, :, t, :]))
        mask_parts.append(np.repeat(mask[:, :, t, :Tt], 16, axis=1))
        sb = seed_blocks[t]
        seed_parts.append(np.repeat(
            sb.reshape(NC, G, -1), 16, axis=1))
    idxs = np.ascontiguousarray(np.concatenate(idx_parts, axis=2))
    maskw = np.ascontiguousarray(np.concatenate(mask_parts, axis=2))
    seedw = np.ascontiguousarray(np.concatenate(seed_parts, axis=2))

    per_core = [{"idxs": idxs[c], "maskp": maskw[c], "seedp": seedw[c]}
                for c in range(NC)]
    return T_list, per_core


# ------------------------------------------------------------- device build

def build_nc(cfg, T_list, max_waits=2, ctrl_max_waits=1, split=True):
    NC, N, F, H = cfg["NCORES"], cfg["N"], cfg["F"], cfg["H"]
    NL, NB, NT = cfg["NL"], cfg["NB"], cfg["NT"]
    CH, NCH = cfg["CH"], cfg["NCH"]
    KB = F // 128
    NLP = NL + 16               # table width incl. sentinel
    T_max = max(T_list)
    # per-tile offsets into the concatenated idx / mask / seed streams
    idx_off, mask_off, seed_off = [], [], []
    io = mo = so = 0
    for t in range(NT):
        Tt = T_list[t]
        idx_off.append(io); mask_off.append(mo); seed_off.append(so)
        io += Tt // 16 + 32
        mo += Tt
        so += _halves(Tt)[0] * 512
    IWT, MWT, SWT = io, mo, so
    NBLK = math.ceil(NL / 128)
    NL2 = NBLK * 128
    W2 = NL2 // 2               # fp32 words backing the bf16 out1T row
    rg = [list(range(NC))]

    nc = bacc.Bacc("TRN2", target_bir_lowering=False)

    # GPSIMD ucode library fence: keep local_scatter (lib 7) and ap_gather
    # (lib 6) phases contiguous in the Pool engine stream so Bacc inserts
    # only one IRAM reload per phase switch, without fencing other engines.
    _fence = {"last": None, "lib": None}

    def _ucode(inst, lib):
        if _fence["last"] is not None and _fence["lib"] != lib:
            add_dep_helper(inst.ins, _fence["last"].ins, sync=False,
                           reason="gpsimd lib phase fence")
        _fence["last"], _fence["lib"] = inst, lib
        return inst

    xtw = nc.declare_dram_parameter("xtw", [128, NCH * KB * CH], BF16, isOutput=False)
    w1 = nc.declare_dram_parameter("w1", [F, H], F32, isOutput=False)
    w2 = nc.declare_dram_parameter("w2", [H, H], F32, isOutput=False)
    a1rep = nc.declare_dram_parameter("a1rep", [H, 16], F32, isOutput=False)
    ad1rep = nc.declare_dram_parameter("ad1rep", [H, 16], F32, isOutput=False)
    a2rep = nc.declare_dram_parameter("a2rep", [H, 16], F32, isOutput=False)
    ad2rep = nc.declare_dram_parameter("ad2rep", [H, 16], F32, isOutput=False)
    b1p = nc.declare_dram_parameter("b1p", [H, 1], F32, isOutput=False)
    b2p = nc.declare_dram_parameter("b2p", [H, 1], F32, isOutput=False)
    selp = nc.declare_dram_parameter("selp", [128, 16], F32, isOutput=False)
    identp = nc.declare_dram_parameter("identp", [16, 16], F32, isOutput=False)
    idxsp = nc.declare_dram_parameter("idxs", [128, IWT], I16, isOutput=False)
    maskp = nc.declare_dram_parameter("maskp", [128, MWT], F16, isOutput=False)
    seedp = nc.declare_dram_parameter("seedp", [128, SWT], I16, isOutput=False)
    outp = nc.declare_dram_parameter("out", [128, NBLK * H], F32, isOutput=True)

    ag_in = [nc.dram_tensor(f"ag_in{l}", [16, NL], F32) for l in (1, 2)]
    ag_out = [nc.dram_tensor(f"ag_out{l}", [128, NL], F32, addr_space="Shared")
              for l in (1, 2)]
    # per-node fp16 alpha_dst rows (own chunk), incl. zero pad wide enough
    # for the last tile's 512-wide adbt broadcast read
    NLP2 = max(NLP, (NT - 1) * NB + 512)
    ad_row = [nc.dram_tensor(f"ad_row{l}", [1, NLP2], F16) for l in (1, 2)]
    # per-edge alpha_dst streams staged by the seed phase
    adeg = [nc.dram_tensor(f"adeg{l}", [128, MWT], F16) for l in (1, 2)]

    with tile.TileContext(nc, num_cores=NC) as tc:
        with tc.tile_pool(name="const", bufs=1) as cpool:
            w1t = cpool.tile([128, KB, H], BF16)
            nc.gpsimd.dma_start(out=w1t[:], in_=w1[:].rearrange("(b p) h -> p b h", p=128))
            # w2 / identity copies aligned to the partitions where out1T /
            # out2T live (matmul needs lhsT and rhs on the same partitions).
            w2t = cpool.tile([48, H], BF16)
            nc.gpsimd.dma_start(out=w2t[32:48, :], in_=w2[:])
            a1t = cpool.tile([16, 16], BF16)
            nc.gpsimd.dma_start(out=a1t[:], in_=a1rep[:])
            ad1t = cpool.tile([16, 16], BF16)
            nc.gpsimd.dma_start(out=ad1t[:], in_=ad1rep[:])
            a2t = cpool.tile([16, 16], BF16)
            nc.gpsimd.dma_start(out=a2t[:], in_=a2rep[:])
            ad2t = cpool.tile([16, 16], BF16)
            nc.gpsimd.dma_start(out=ad2t[:], in_=ad2rep[:])
            b1t = cpool.tile([16, 1], F32)
            nc.sync.dma_start(out=b1t[:], in_=b1p[:])
            b2t = cpool.tile([16, 1], F32)
            nc.sync.dma_start(out=b2t[:], in_=b2p[:])
            selt = cpool.tile([128, 16], F16)
            nc.gpsimd.dma_start(out=selt[:], in_=selp[:])
            idt = cpool.tile([80, 16], F32)
            nc.sync.dma_start(out=idt[64:80, :], in_=identp[:])

            # Stacked per-layer state, one tile so 16-partition rows share
            # column space; engine accesses must start at partition 0/32/64/96:
            # p0-15 wself, p32-47 out1T (bf16, matmul rhs -> base 32),
            # p64-79 out2T (transpose input -> base 64), p96-111 pself.
            stk = cpool.tile([128, NL2], F32)
            wselfv = stk[0:16, 0:NL]
            pselfv = stk[96:112, 0:NL]
            out1v = stk[32:48, 0:W2].bitcast(BF16)      # [16, NL2] bf16
            out2v = stk[64:80, 0:NL2]
            nc.vector.memset(stk[64:80, NL:], 0.0)

            zero16 = cpool.tile([1, NLP2 - NL], F16)
            nc.vector.memset(zero16[:], 0.0)

            idxs_sb = cpool.tile([128, IWT], I16)
            nc.gpsimd.dma_start(out=idxs_sb[:], in_=idxsp[:])

            # ---------------- phase A (layer 1): tables from x^T ----------
            with (
                tc.tile_pool(name="pa", bufs=2) as pa,
                tc.tile_pool(name="pap", bufs=2, space="PSUM") as pap,
            ):
                for c in range(NCH):
                    sl = slice(c * CH, (c + 1) * CH)
                    xt_t = pa.tile([128, KB, CH], BF16, tag="xt")
                    nc.sync.dma_start(
                        out=xt_t[:],
                        in_=xtw[:, c * KB * CH:(c + 1) * KB * CH]
                        .rearrange("p (b n) -> p b n", b=KB))
                    ph = pap.tile([16, CH], F32, tag="ph")
                    for b in range(KB):
                        nc.tensor.matmul(ph[:], lhsT=w1t[:, b, :], rhs=xt_t[:, b, :],
                                         start=(b == 0), stop=(b == KB - 1))
                    hch = pa.tile([16, CH], BF16, tag="hch")
                    nc.scalar.copy(hch[:], ph[:])
                    pas = pap.tile([16, CH], F32, tag="pas")
                    nc.tensor.matmul(pas[:], lhsT=a1t[:], rhs=hch[:], start=True, stop=True)
                    pad_ = pap.tile([16, CH], F32, tag="pad")
                    nc.tensor.matmul(pad_[:], lhsT=ad1t[:], rhs=hch[:], start=True, stop=True)
                    packed = pa.tile([16, CH], F32, tag="packed")
                    pk = packed[:].bitcast(F16)
                    nc.vector.tensor_copy(pk[:, 0::2], hch[:])
                    nc.vector.tensor_copy(pk[:, 1::2], pas[:])
                    nc.sync.dma_start(out=ag_in[0][:, sl], in_=packed[:])
                    adrow = pa.tile([1, CH], F16, tag="adrow")
                    nc.vector.tensor_copy(adrow[:], pad_[0:1, :])
                    nc.sync.dma_start(out=ad_row[0][0:1, sl], in_=adrow[:])
                    # analytic self-loop contribution for this chunk
                    adfull = pa.tile([16, CH], F32, tag="adfull")
                    nc.scalar.copy(adfull[:], pad_[:])
                    tself = pa.tile([16, CH], F32, tag="tself")
                    nc.vector.tensor_add(tself[:], pas[:], adfull[:])
                    nc.vector.scalar_tensor_tensor(
                        tself[:], tself[:], NEG_SLOPE, tself[:],
                        op0=mybir.AluOpType.mult, op1=mybir.AluOpType.max)
                    pselfc = pa.tile([16, CH], F32, tag="pselfc")
                    nc.scalar.activation(pselfc[:], tself[:],
                                         mybir.ActivationFunctionType.Exp)
                    nc.scalar.copy(pselfv[:, sl], pselfc[:])
                    nc.vector.tensor_mul(wselfv[:, sl], pselfc[:], hch[:])

            def run_layer(l, writer):
                nc.sync.dma_start(out=ad_row[l][0:1, NL:], in_=zero16[:])
                nc.gpsimd.collective_compute(
                    "AllGather", mybir.AluOpType.bypass, replica_groups=rg,
                    ins=[ag_in[l][:]], outs=[ag_out[l][:]])
                # ---- seed phase: alpha_dst -> run-start seeds -> scan ----
                with tc.tile_pool(name=f"sd{l}", bufs=4) as sd:
                    for t in range(NT):
                        Tt = T_list[t]
                        NHt, W0t = _halves(Tt)
                        adbt = sd.tile([128, 512], F16, tag="adbt")
                        nc.sync.dma_start(
                            out=adbt[:],
                            in_=ad_row[l][0:1, t * NB:t * NB + 512]
                            .to_broadcast([128, 512]))
                        sidx = sd.tile([128, 512 * (T_max // LS_MAX + 1)],
                                       I16, tag="sidx")
                        nc.sync.dma_start(
                            out=sidx[:, :NHt * 512],
                            in_=seedp[:, seed_off[t]:seed_off[t] + NHt * 512])
                        seeds = sd.tile([128, T_max], F16, tag="seeds")
                        for h in range(NHt):
                            w0, w1_ = h * W0t, min((h + 1) * W0t, Tt)
                            _ucode(nc.gpsimd.local_scatter(
                                seeds[:, w0:w1_], adbt[:],
                                sidx[:, h * 512:(h + 1) * 512],
                                channels=128, num_elems=w1_ - w0,
                                num_idxs=512), "ls")
                        msk = sd.tile([128, T_max], F16, tag="msk")
                        nc.sync.dma_start(
                            out=msk[:, :Tt],
                            in_=maskp[:, mask_off[t]:mask_off[t] + Tt])
                        ade = sd.tile([128, T_max], F16, tag="ade")
                        nc.vector.tensor_tensor_scan(
                            ade[:, :Tt], msk[:, :Tt], seeds[:, :Tt], 0.0,
                            mybir.AluOpType.mult, mybir.AluOpType.add)
                        nc.sync.dma_start(
                            out=adeg[l][:, mask_off[t]:mask_off[t] + Tt],
                            in_=ade[:, :Tt])
                # ---- edge phase: h|as gather + per-edge math + seg-sums ----
                with tc.tile_pool(name=f"tab{l}", bufs=1) as tabp:
                    table = tabp.tile([128, NLP], F32, tag="table")
                    nc.sync.dma_start(out=table[:, :NL], in_=ag_out[l][:])
                    nc.vector.memset(table[:, NL:], SENT_PACKED)
                    with (
                        tc.tile_pool(name=f"ed{l}", bufs=2) as ed,
                        tc.tile_pool(name=f"eb{l}", bufs=3) as eb,
                        tc.tile_pool(name=f"e1{l}", bufs=1) as e1,
                        tc.tile_pool(name=f"edp{l}", bufs=2, space="PSUM") as edp,
                    ):
                        # software-pipelined: issue tile t's h|as gather one
                        # iteration ahead of its per-edge math, so the DVE
                        # chain of tile t hides under the gather of t+1 and
                        # GPSIMD stays busy back-to-back.
                        pend = {}

                        def issue(t):
                            Tt = T_list[t]
                            o = idx_off[t]
                            ghs = ed.tile([128, T_max], F32, tag="ghs")
                            _ucode(nc.gpsimd.ap_gather(
                                ghs[:, :Tt], table[:],
                                idxs_sb[:, o:o + Tt // 16],
                                channels=128, num_elems=NLP, d=1,
                                num_idxs=Tt), "ag")
                            ade = e1.tile([128, T_max], F16, tag="adg", bufs=3)
                            nc.sync.dma_start(
                                out=ade[:, :Tt],
                                in_=adeg[l][:, mask_off[t]:mask_off[t] + Tt])
                            msk = e1.tile([128, T_max], F16, tag="mask", bufs=3)
                            nc.sync.dma_start(
                                out=msk[:, :Tt],
                                in_=maskp[:, mask_off[t]:mask_off[t] + Tt])
                            pend[t] = (ghs, ade, msk)

                        def process(t):
                            ghs, ade, msk = pend.pop(t)
                            Tt = T_list[t]
                            o = idx_off[t]
                            i_bnd = idxs_sb[:, o + Tt // 16:o + Tt // 16 + 32]
                            g16 = ghs[:, :Tt].bitcast(F16)   # [128, 2*Tt]
                            tt = ed.tile([128, T_max], F16, tag="tt")
                            nc.vector.tensor_add(tt[:, :Tt], g16[:, 1::2], ade[:, :Tt])
                            nc.vector.scalar_tensor_tensor(
                                tt[:, :Tt], tt[:, :Tt], NEG_SLOPE, tt[:, :Tt],
                                op0=mybir.AluOpType.mult, op1=mybir.AluOpType.max)
                            pp = e1.tile([128, T_max], F16, tag="pp", bufs=2)
                            nc.scalar.activation(pp[:, :Tt], tt[:, :Tt],
                                                 mybir.ActivationFunctionType.Exp)
                            ww = ed.tile([128, T_max], F16, tag="ww")
                            nc.vector.tensor_mul(ww[:, :Tt], pp[:, :Tt], g16[:, 0::2])
                            sc = eb.tile([128, T_max], F32, tag="sc", bufs=2)
                            sc16 = sc[:, :Tt].bitcast(F16)   # (w | p) lanes
                            nc.vector.tensor_tensor_scan(
                                sc16[:, 0::2], msk[:, :Tt], ww[:, :Tt], 0.0,
                                mybir.AluOpType.mult, mybir.AluOpType.add)
                            nc.vector.tensor_tensor_scan(
                                sc16[:, 1::2], msk[:, :Tt], pp[:, :Tt], 0.0,
                                mybir.AluOpType.mult, mybir.AluOpType.add)
                            bg = e1.tile([128, 512], F32, tag="bg", bufs=2)
                            _ucode(nc.gpsimd.ap_gather(
                                bg[:], sc[:, :Tt], i_bnd,
                                channels=128, num_elems=Tt, d=1,
                                num_idxs=512), "ag")
                            bg16 = bg[:].bitcast(F16)
                            pu = edp.tile([16, 512], F32, tag="pu")
                            nc.tensor.matmul(pu[:], lhsT=selt[:], rhs=bg16[:, 0::2],
                                             start=True, stop=True)
                            ps = edp.tile([16, 512], F32, tag="ps")
                            nc.tensor.matmul(ps[:], lhsT=selt[:], rhs=bg16[:, 1::2],
                                             start=True, stop=True)
                            # add analytic self-loop terms, then normalize
                            den = ed.tile([16, NB], F32, tag="s16", bufs=4)
                            nc.vector.tensor_add(den[:], ps[:, :NB],
                                                 pselfv[:, t * NB:(t + 1) * NB])
                            rc = ed.tile([16, NB], F32, tag="s16", bufs=4)
                            nc.vector.reciprocal_approx_fast(out=rc[:], in_=den[:])
                            num = ed.tile([16, NB], F32, tag="s16", bufs=4)
                            nc.vector.tensor_add(num[:], pu[:, :NB],
                                                 wselfv[:, t * NB:(t + 1) * NB])
                            uv = ed.tile([16, NB], F32, tag="s16", bufs=4)
                            nc.vector.tensor_mul(uv[:], num[:], rc[:])
                            writer(t, uv)

                        issue(0)
                        for t in range(1, NT):
                            issue(t)
                            process(t - 1)
                        process(NT - 1)

            def write1(t, uv):
                nc.scalar.activation(out1v[:, t * NB:(t + 1) * NB], uv[:],
                                     mybir.ActivationFunctionType.Relu,
                                     bias=b1t[:, 0:1])

            def write2(t, uv):
                nc.vector.tensor_scalar_add(out2v[:, t * NB:(t + 1) * NB], uv[:],
                                            b2t[:, 0:1])

            run_layer(0, write1)

            # ---------------- phase A (layer 2): tables from out1T --------
            with (
                tc.tile_pool(name="pa2", bufs=2) as pa2,
                tc.tile_pool(name="pap2", bufs=2, space="PSUM") as pap2,
            ):
                for c in range(NCH):
                    sl = slice(c * CH, (c + 1) * CH)
                    ph = pap2.tile([16, CH], F32, tag="ph2")
                    nc.tensor.matmul(ph[:], lhsT=w2t[32:48, :], rhs=out1v[:, sl],
                                     start=True, stop=True)
                    h2ch = pa2.tile([16, CH], BF16, tag="h2ch")
                    nc.scalar.copy(h2ch[:], ph[:])
                    pas = pap2.tile([16, CH], F32, tag="pas2")
                    nc.tensor.matmul(pas[:], lhsT=a2t[:], rhs=h2ch[:], start=True, stop=True)
                    pad_ = pap2.tile([16, CH], F32, tag="pad2")
                    nc.tensor.matmul(pad_[:], lhsT=ad2t[:], rhs=h2ch[:], start=True, stop=True)
                    packed = pa2.tile([16, CH], F32, tag="packed2")
                    pk = packed[:].bitcast(F16)
                    nc.vector.tensor_copy(pk[:, 0::2], h2ch[:])
                    nc.vector.tensor_copy(pk[:, 1::2], pas[:])
                    nc.sync.dma_start(out=ag_in[1][:, sl], in_=packed[:])
                    adrow = pa2.tile([1, CH], F16, tag="adrow2")
                    nc.vector.tensor_copy(adrow[:], pad_[0:1, :])
                    nc.sync.dma_start(out=ad_row[1][0:1, sl], in_=adrow[:])
                    adfull = pa2.tile([16, CH], F32, tag="adfull2")
                    nc.scalar.copy(adfull[:], pad_[:])
                    tself = pa2.tile([16, CH], F32, tag="tself2")
                    nc.vector.tensor_add(tself[:], pas[:], adfull[:])
                    nc.vector.scalar_tensor_tensor(
                        tself[:], tself[:], NEG_SLOPE, tself[:],
                        op0=mybir.AluOpType.mult, op1=mybir.AluOpType.max)
                    pselfc = pa2.tile([16, CH], F32, tag="pselfc2")
                    nc.scalar.activation(pselfc[:], tself[:],
                                         mybir.ActivationFunctionType.Exp)
                    nc.scalar.copy(pselfv[:, sl], pselfc[:])
                    nc.vector.tensor_mul(wselfv[:, sl], pselfc[:], h2ch[:])

            run_layer(1, write2)

            # ---------------- log_softmax + transpose + store -------------
            with (
                tc.tile_pool(name="fin", bufs=2) as fin,
                tc.tile_pool(name="finp", bufs=2, space="PSUM") as finp,
                tc.tile_pool(name="fino", bufs=1) as fino,
            ):
                nodemaj = fino.tile([128, NBLK, H], F32, tag="nodemaj")
                for r in range(0, NBLK, 16):
                    nb = min(16, NBLK - r)
                    ptp = finp.tile([128, 16, 16], F32, tag="ptp")
                    for j in range(nb):
                        nc.tensor.transpose(ptp[:, j, :],
                                            out2v[:, (r + j) * 128:(r + j + 1) * 128],
                                            idt[64:80, :])
                    nc.vector.tensor_copy(nodemaj[:, r:r + nb, :], ptp[:, 0:nb, :])
                mx = fin.tile([128, NBLK], F32, tag="mx")
                nc.vector.tensor_reduce(mx[:], nodemaj[:], axis=mybir.AxisListType.X,
                                        op=mybir.AluOpType.max)
                zz = fino.tile([128, NBLK, H], F32, tag="zz")
                nc.vector.tensor_sub(zz[:], nodemaj[:],
                                     mx[:, :, None].to_broadcast([128, NBLK, H]))
                es = fino.tile([128, NBLK, H], F32, tag="es")
                nc.scalar.activation(es[:], zz[:], mybir.ActivationFunctionType.Exp)
                sm = fin.tile([128, NBLK], F32, tag="sm")
                nc.vector.tensor_reduce(sm[:], es[:], axis=mybir.AxisListType.X,
                                        op=mybir.AluOpType.add)
                ls = fin.tile([128, NBLK], F32, tag="ls")
                nc.scalar.activation(ls[:], sm[:], mybir.ActivationFunctionType.Ln)
                outf = fino.tile([128, NBLK, H], F32, tag="outf")
                nc.vector.tensor_sub(outf[:], zz[:],
                                     ls[:, :, None].to_broadcast([128, NBLK, H]))
                nc.gpsimd.dma_start(out=outp[:].rearrange("p (b h) -> p b h", h=H),
                                    in_=outf[:])

    nc.compile()
    n_reload = sum(
        1
        for f in nc.m.functions
        for bb in f.blocks
        for ins in bb.instructions
        if type(ins).__name__ == "InstPseudoReloadLibraryIndex"
    )
    assert n_reload <= 8, f"library thrash: {n_reload} reloads"
    if split:
        split_waits(nc, max_waits=max_waits, ctrl_max_waits=ctrl_max_waits)
    return nc


CTRL_TYPES = ("InstDrain", "InstNoOp", "InstHalt", "InstEventSemaphore")


def split_waits(nc, max_waits=2, ctrl_max_waits=1):
    """walrus in this container caps sync-waits per instruction; move excess
    waits onto preceding same-engine NoOps (each carrying one wait)."""
    for f in nc.m.functions:
        for bb in f.blocks:
            new_insts, changed = [], False
            for ins in bb.instructions:
                si = ins.sync_info
                cap = (ctrl_max_waits if type(ins).__name__ in CTRL_TYPES
                       else max_waits)
                if si is not None and si.on_wait is not None and len(si.on_wait) > cap:
                    waits = list(si.on_wait)
                    excess, keep = waits[:-cap] if cap else waits, waits[-cap:] if cap else []
                    for i, w in enumerate(excess):
                        nop = mybir.InstNoOp(name=f"{ins.name}-ws{i}", ins=[], outs=[])
                        nop.engine = ins.engine
                        nop.sync_info = mybir.SyncInfo(on_wait=[w], on_update=[])
                        new_insts.append(nop)
                    si.on_wait = keep
                    changed = True
                new_insts.append(ins)
            if changed:
                bb.instructions = new_insts
    # verify the rewrite stuck (pyo3 lists can copy-on-read)
    for f in nc.m.functions:
        for bb in f.blocks:
            for ins in bb.instructions:
                si = ins.sync_info
                cap = (ctrl_max_waits if type(ins).__name__ in CTRL_TYPES
                       else max_waits)
                assert si is None or si.on_wait is None or len(si.on_wait) <= cap, \
                    f"{ins.name}: {len(si.on_wait)} waits > {cap}"


# ------------------------------------------------------------ input packing

def make_in_maps(inputs, cfg, per_core):
    NC, NL, H, F = cfg["NCORES"], cfg["NL"], cfg["H"], cfg["F"]
    NB, CH, NCH = cfg["NB"], cfg["CH"], cfg["NCH"]
    KB = F // 128
    x = np.asarray(inputs["x"], dtype=np.float32)
    sel = np.zeros((128, 16), dtype=np.float32)
    sel[np.arange(128), np.arange(128) % 16] = 1.0
    shared = {
        "w1": np.ascontiguousarray(np.asarray(inputs["W1"], np.float32)),
        "w2": np.ascontiguousarray(np.asarray(inputs["W2"], np.float32)),
        "a1rep": np.ascontiguousarray(np.repeat(np.asarray(inputs["a_src1"], np.float32)[:, None], 16, 1)),
        "ad1rep": np.ascontiguousarray(np.repeat(np.asarray(inputs["a_dst1"], np.float32)[:, None], 16, 1)),
        "a2rep": np.ascontiguousarray(np.repeat(np.asarray(inputs["a_src2"], np.float32)[:, None], 16, 1)),
        "ad2rep": np.ascontiguousarray(np.repeat(np.asarray(inputs["a_dst2"], np.float32)[:, None], 16, 1)),
        "b1p": np.ascontiguousarray(np.asarray(inputs["b1"], np.float32)[:, None]),
        "b2p": np.ascontiguousarray(np.asarray(inputs["b2"], np.float32)[:, None]),
    }
    shared.update({
        "selp": sel,
        "identp": np.eye(16, dtype=np.float32),
    })
    in_maps = []
    for c in range(NC):
        m = dict(shared)
        xpart = x[c * NL:(c + 1) * NL]                    # [NL, F]
        xtw = (xpart.T.reshape(KB, 128, NCH, CH)
               .transpose(1, 2, 0, 3).reshape(128, NCH * KB * CH))
        m["xtw"] = np.ascontiguousarray(xtw.astype(ml_dtypes.bfloat16))
        m["idxs"] = per_core[c]["idxs"]
        m["maskp"] = per_core[c]["maskp"]
        m["seedp"] = per_core[c]["seedp"]
        in_maps.append(m)
    return in_maps


def unshard_output(results, cfg):
    NC, NL, H = cfg["NCORES"], cfg["NL"], cfg["H"]
    NBLK = math.ceil(NL / 128)
    parts = []
    for c in range(NC):
        a = np.asarray(results[c]["out"]).reshape(128, NBLK, H)
        a = a.transpose(1, 0, 2).reshape(NBLK * 128, H)[:NL]
        parts.append(a)
    return np.concatenate(parts, axis=0)


# ------------------------------------------------------------------- driver

_CACHE = {}


def run_on_hw(inputs, cfg, trace=False, tmpdir=None):
    import os
    import shutil
    from concourse.bass_utils import run_bass_kernel_spmd
    if tmpdir is not None and os.path.isdir(tmpdir):
        shutil.rmtree(tmpdir, ignore_errors=True)
    if tmpdir is not None:
        os.makedirs(tmpdir, exist_ok=True)
    T_list, per_core = host_prep(inputs["edge_index"], cfg)
    key = (cfg["N"], tuple(T_list))
    if key not in _CACHE:
        _CACHE[key] = build_nc(cfg, T_list)
    nc = _CACHE[key]
    in_maps = make_in_maps(inputs, cfg, per_core)
    res = run_bass_kernel_spmd(nc, in_maps, list(range(cfg["NCORES"])),
                               trace=trace, tmpdir=tmpdir)
    out = unshard_output(res.results, cfg)
    return out, res


def kernel(**inputs):
    out, _ = run_on_hw(inputs, FULL_CFG)
    return out.astype(np.float32)
